# revision 51
# baseline (speedup 1.0000x reference)
"""Trainium2 Bass kernel for nn_Discriminator_AddDim_ESSAAttn.

Network (per sample, C=128, 27x27 spatial, N=729 tokens):
  ESSA linear attention -> concat -> 1x1-conv FFN (+residual) ->
  3x3 conv/relu/pool x2 -> 3 FC layers -> [16] logits.
Batch 256 is sharded 32-per-core across 8 NeuronCores (pure data
parallel, weights replicated).

Key algebraic folds (vs the straightforward lowering):
  - q2 row-normalisation: q2/(sum+eps) then L2-normalise == q2/||q2||_2
    (the sum cancels), so only sq4 = sum(q2^2) is needed per token.
  - attn = (v+t2) @ w_ln + b_ln is consumed ONLY by ffn1, so the whole
    attn stage folds into ffn1's weights: with WT = w_ln @ w1a,
      h = lrelu(WA^T x + WT^T (v_cm + t2_cm) + b1')
      WA = w1x + wv @ WT,  b1' = b1 + w1a^T b_ln + WT^T b_v.
  - the WT^T t2 product re-associates: WT^T (kv^T q2nT) = M^T q2nT with
    M = k2a^T (v @ WT).  v @ WT folds into the qkv weights (v-slot of
    wqkv becomes the 64-wide vW = wv @ WT slot), and M comes out of the
    same PE pass as the k2a gram (for the column norms).  The t2 psum
    stage, its 729-col extract, and the 128-col v extract all vanish.
  - the k2a column norm (invs) applies per-PARTITION on the transposed
    q2nT, so it rides the q2nT psum->sbuf extract for free.
All ESSA-chain matmuls run in bf16 (1 cyc/col on the PE, no fp32r
<256-col penalty, DVE 2x/4x fast modes on the extracts).
"""
import sys

sys.path.insert(0, "/opt/trn_rl_repo")

import numpy as np

import concourse.bass as bass
import concourse.tile as tile
from concourse import mybir
from concourse.bass_utils import run_bass_kernel_spmd

F32 = mybir.dt.float32
F32R = mybir.dt.float32r
BF16 = mybir.dt.bfloat16
AF = mybir.ActivationFunctionType
ALU = mybir.AluOpType
AX = mybir.AxisListType

N_CORES = 8
B, C, P = 256, 128, 27
NTOK = P * P          # 729
S = B // N_CORES      # 32 samples per core
NT = 6                # token tiles: 5*128 + 89
TOK_SIZES = [128, 128, 128, 128, 128, 89]
CGRP = 4              # conv2 sample-group size
QW = 320              # qkv output width: q(128) | k(128) | vW(64)


def _split_waits(nc, maxw=1):
    """walrus CoreV3 rejects instructions carrying >1 sem-wait; hoist
    extras onto preceding same-engine no-op carriers."""
    import bass_rust

    for bb in nc.m.functions[0].blocks:
        newlist = []
        for ins in bb.instructions:
            sw = ins.sync_info
            if sw and sw.on_wait and len(sw.on_wait) > maxw:
                waits = list(sw.on_wait)
                keep = waits[-maxw:]
                hoist = waits[:-maxw]
                for i in range(0, len(hoist), maxw):
                    chunk = hoist[i : i + maxw]
                    nop = bass_rust.InstNoOp(
                        name=f"{ins.name}_wsplit{i}", ins=[], outs=[]
                    )
                    nop.engine = ins.engine
                    nop.sync_info = mybir.SyncInfo(on_wait=list(chunk), on_update=[])
                    nc.register_instruction(nop, overwrite=True)
                    newlist.append(nop)
                ins.sync_info = mybir.SyncInfo(
                    on_wait=list(keep), on_update=list(sw.on_update)
                )
            newlist.append(ins)
        bb.instructions[:] = newlist


def _prep_weights(inputs):
    """Host-side weight massaging (all cheap numpy)."""
    f = lambda a: np.ascontiguousarray(np.asarray(a, np.float32))
    w_qkv = f(inputs["w_qkv"]).copy()          # [128, 384]
    b_qkv = f(inputs["b_qkv"]).copy()          # [384]
    # fold channel-mean subtraction of q and k into the weights/bias
    w_qkv[:, 0:128] -= w_qkv[:, 0:128].mean(axis=1, keepdims=True)
    w_qkv[:, 128:256] -= w_qkv[:, 128:256].mean(axis=1, keepdims=True)
    b_qkv[0:128] -= b_qkv[0:128].mean()
    b_qkv[128:256] -= b_qkv[128:256].mean()
    wv = w_qkv[:, 256:384]                     # [128, 128]
    bv = b_qkv[256:384]

    w_ln = f(inputs["w_ln"])                   # [128, 128]
    b_ln = f(inputs["b_ln"])                   # [128]
    w_ffn1 = f(inputs["w_ffn1"]).reshape(64, 256)     # [out, in]
    w1x = w_ffn1[:, 0:128].T                   # [128, 64]
    w1a = w_ffn1[:, 128:256].T                 # [128, 64]
    WT = w_ln @ w1a                            # [128, 64]
    WA = w1x + wv @ WT                         # [128, 64]
    b1p = f(inputs["b_ffn1"]) + w1a.T @ b_ln + WT.T @ bv   # [64]

    # extended qkv: q | k | vW, with vW = x^T (wv @ WT)
    wqkv_ext = np.concatenate([w_qkv[:, 0:256], wv @ WT], axis=1)  # [128, 320]
    bqkv_ext = np.concatenate([b_qkv[0:256], WT.T @ bv])           # [320]

    w2t1 = f(inputs["w_ffn2"]).reshape(128, 64).T          # [64, 128]
    # duplicated row-block: rows 64-127 serve the partition-stacked pair
    # sample (its h lives on sbuf partitions 64-127)
    w2t = np.ascontiguousarray(np.concatenate([w2t1, w2t1], axis=0))  # [128, 128]

    # conv taps -> [in_ch, 9, out_ch]
    wc1 = np.ascontiguousarray(
        f(inputs["w_c1"]).transpose(2, 3, 1, 0).reshape(9, 128, 64).transpose(1, 0, 2)
    )  # [128, 9, 64]
    wc2 = np.ascontiguousarray(
        f(inputs["w_c2"]).transpose(2, 3, 1, 0).reshape(9, 64, 128).transpose(1, 0, 2)
    )  # [64, 9, 128]

    w1r = np.ascontiguousarray(f(inputs["w_fc1"]).reshape(128, 25, 512))
    wf2 = np.ascontiguousarray(f(inputs["w_fc2"]).reshape(4, 128, 512).transpose(1, 0, 2))
    wcls = np.ascontiguousarray(f(inputs["w_cls"]).reshape(4, 128, 16).transpose(1, 0, 2))

    col = lambda a: np.ascontiguousarray(np.asarray(a, np.float32).reshape(-1, 1))
    row = lambda a: np.ascontiguousarray(np.asarray(a, np.float32).reshape(1, -1))
    w = {
        "wqkv": np.ascontiguousarray(wqkv_ext),
        "bqkv_row": row(bqkv_ext),
        "WA": np.ascontiguousarray(WA),
        "b1": col(np.concatenate([b1p, b1p])),
        "w2t": w2t,
        "b2": col(inputs["b_ffn2"]),
        "wc1": wc1,
        "bc1": col(inputs["b_c1"]),
        "wc2": wc2,
        "bc2": col(inputs["b_c2"]),
        "w1r": w1r,
        "b1row": row(inputs["b_fc1"]),
        "wf2": wf2,
        "b2row": row(inputs["b_fc2"]),
        "wcls": wcls,
        "bcrow": row(inputs["b_cls"]),
        "eye": np.eye(128, dtype=np.float32),
        "eyebf": np.eye(128, dtype=np.float32),
        "ones1": np.ones((1, S), dtype=np.float32),
    }
    flags = {
        "qkv_bias": bool(np.any(bqkv_ext)),
        "fc1_bias": bool(np.any(w["b1row"])),
        "fc2_bias": bool(np.any(w["b2row"])),
        "cls_bias": bool(np.any(w["bcrow"])),
    }
    return w, flags


class _W:
    pass


_F32_WEIGHTS = {"b1", "b2", "bc1", "bc2"}            # activation-bias operands
_BF16_WEIGHTS = {"wqkv", "WA", "w2t", "wc1", "wc2", "eyebf"}  # bf16 matmul path


def _load_weights(nc, pool, wvals):
    """Declare dram params + DMA every weight into resident SBUF tiles.
    bf16 weights are cast during a gpsimd DMA (only engine that casts)."""
    W = _W()
    for name, arr in wvals.items():
        if name in _F32_WEIGHTS:
            dt = F32
        elif name in _BF16_WEIGHTS:
            dt = BF16
        else:
            dt = F32R
        dram = nc.declare_dram_parameter(
            name, list(arr.shape), F32 if dt == BF16 else dt, isOutput=False
        )
        t = pool.tile(list(arr.shape), dt, name=f"sb_{name}")
        if dt == BF16:
            nc.gpsimd.dma_start(out=t, in_=dram[:])
        else:
            nc.sync.dma_start(out=t, in_=dram[:])
        setattr(W, name, t)
    return W


def _win(ap, offset, dims):
    """Manual sub-AP of a tile: dims = [[stride, count], ...] free dims."""
    return bass.AP(
        tensor=ap.tensor, offset=ap.offset + offset,
        ap=[list(ap.ap[0])] + [list(d) for d in dims],
    )


def _bcast(ap, nt, width):
    """[128, NT] stat tile -> broadcast AP [128, nt, width] (0-stride inner)."""
    return bass.AP(
        tensor=ap.tensor, offset=ap.offset,
        ap=[list(ap.ap[0]), [1, nt], [0, width]],
    )


def _mm(nc, out, lhsT, rhs, start=True, stop=True):
    nc.tensor.matmul(out, lhsT, rhs, start=start, stop=stop)


def _tp(nc, out, in_, eye):
    nc.tensor.matmul(out, in_, eye, is_transpose=True)


def _s0_start(nc, pools, x_dram, s):
    """Allocate the sample's tiles + launch its x DMA."""
    acts = pools["acts"]
    st = {"s": s}
    x_s = acts.tile([C, 768], BF16, name="x_s", bufs=5)
    nc.sync.dma_start(out=x_s[:, 0:NTOK], in_=x_dram[s])
    st["x_s"] = x_s
    st["q2k2"] = acts.tile([128, NT, 256], BF16, name="q2k2", bufs=4)
    st["vwr"] = acts.tile([128, NT, 64], BF16, name="vwr", bufs=4)
    return st


def _s0_qkv(nc, pools, W, flags, st):
    """qkv in two psum halves; extracts per half (ACT square + DVE vW)."""
    psum = pools["psum"]
    x_s, q2k2, vwr = st["x_s"], st["q2k2"], st["vwr"]
    for half in range(2):
        pq = psum.tile([128, 3, 512], F32, name="pq", tag="pqkv", bufs=1)
        for i in range(3):
            t = half * 3 + i
            nt = TOK_SIZES[t]
            _mm(nc, pq[0:nt, i, 0:QW], x_s[:, 128 * t : 128 * t + nt],
                W.wqkv, start=True, stop=not flags["qkv_bias"])
            if flags["qkv_bias"]:
                _mm(nc, pq[0:nt, i, 0:QW], W.ones1[0:1, 0:nt], W.bqkv_row,
                    start=False, stop=True)
        h3 = slice(3 * half, 3 * half + 3)
        nc.scalar.activation(q2k2[:, h3, :], pq[:, :, 0:256], AF.Square)
        nc.vector.tensor_copy(vwr[:, h3, :], pq[:, :, 256:320])


def _s0_stats(nc, pools, W, st):
    """Token stats + q2n / k2a scales (needs both qkv halves)."""
    acts, stats = pools["acts"], pools["stats"]
    q2k2, vwr = st["q2k2"], st["vwr"]
    q2 = q2k2.rearrange("p t (h c) -> p t h c", h=2)[:, :, 0, :]   # [128,NT,128]
    k2 = q2k2.rearrange("p t (h c) -> p t h c", h=2)[:, :, 1, :]

    # wide square: q4k4 = (q2|k2)^2 in one op; q-half feeds sq4, k-half is
    # the k^4 stationary for the 1-col s2 matmul in the kv stage.
    q4k4 = acts.tile([128, NT, 256], BF16, name="q4k4", bufs=4)
    nc.vector.tensor_mul(q4k4, q2k2, q2k2)
    sk2 = stats.tile([128, NT], F32, name="sk2", bufs=3)
    nc.vector.reduce_sum(sk2, k2, axis=AX.X)
    # sk2 = sum_c k^2 ~ O(5) for randn inputs; the reference's +1e-7 is
    # numerically invisible, so skip the add
    s1kb = stats.tile([128, NT], BF16, name="s1kb", bufs=3)
    s1k2 = stats.tile([128, NT], BF16, name="s1k2", bufs=4)
    with nc.allow_low_precision(reason="bf16 per-token scales, ~0.4% on attn"):
        nc.vector.reciprocal(s1kb, sk2)
    nc.vector.tensor_mul(s1k2, s1kb, s1kb)
    # fold s1k into vW instead of scaling k2: M = k2a^T vW == k2^T (vW*s1k)
    vws = acts.tile([128, NT, 64], BF16, name="vws", bufs=4)
    nc.gpsimd.tensor_tensor(vws, vwr, _bcast(s1kb, NT, 64), ALU.mult)
    st.update(vws=vws, q4k4=q4k4, s1k2=s1k2)

    sq4 = stats.tile([128, NT], F32, name="sq4", bufs=3)
    nc.vector.reduce_sum(sq4, q4k4[:, :, 0:128], axis=AX.X)
    nq = stats.tile([128, NT], F32, name="nq", bufs=3)
    nc.scalar.activation(nq, sq4, AF.Sqrt)
    cqb = stats.tile([128, NT], BF16, name="cqb", bufs=3)
    with nc.allow_low_precision(reason="bf16 per-token scales, ~0.4% on attn"):
        nc.vector.reciprocal(cqb, nq)
    q2n = acts.tile([128, NT, 128], BF16, name="q2n", bufs=4)
    nc.gpsimd.tensor_tensor(q2n, q2, _bcast(cqb, NT, 128), ALU.mult)
    st.update(q2n=q2n, q2k2=q2k2)
    return st


def _s1_kv_pair(nc, pools, W, sts, pkv):
    """M = k2a^T vW and the k2a gram (column norms) in one PE pass.
    Both samples share one psum bank; gram-diag/invs chain pair-fused.
    invs lands on the msb extract (per-partition there), so q2nT stays a
    plain copy off the transpose."""
    acts, stats = pools["acts"], pools["stats"]
    npair = len(sts)
    for si, st in enumerate(sts):
        q2k2, vws, q4k4, s1k2 = st["q2k2"], st["vws"], st["q4k4"], st["s1k2"]
        for t in range(NT):
            nt = TOK_SIZES[t]
            _mm(nc, pkv[:, si, 0:64], q2k2[0:nt, t, 128:256], vws[0:nt, t, :],
                start=(t == 0), stop=(t == NT - 1))
            _mm(nc, pkv[:, si, 64:65], q4k4[0:nt, t, 128:256],
                s1k2[0:nt, t : t + 1], start=(t == 0), stop=(t == NT - 1))
    invs = stats.tile([128, 2], F32, name="invs", bufs=3)
    for si, st in enumerate(sts):
        nc.scalar.activation(invs[:, si : si + 1], pkv[:, si, 64:65], AF.Sqrt,
                             scale=float(NTOK))  # 27*sqrt(s2)
    nc.vector.reciprocal(invs[:, 0:npair], invs[:, 0:npair])
    for si, st in enumerate(sts):
        msb = acts.tile([128, 64], BF16, name="msb", bufs=3)
        nc.scalar.mul(msb, pkv[:, si, 0:64], invs[:, si : si + 1])
        st["msb"] = msb


def _s1_tp(nc, pools, W, st):
    """Transpose q2n to channel-major."""
    acts, psum = pools["acts"], pools["psum"]
    q2n = st["q2n"]
    pqt = psum.tile([128, 768], BF16, name="pqt", tag="ps2", bufs=2)
    for t in range(NT):
        _tp(nc, pqt[:, 128 * t : 128 * (t + 1)], q2n[:, t, :], W.eyebf)
    q2nT = acts.tile([128, 768], BF16, name="q2nT", bufs=2)
    nc.scalar.copy(q2nT, pqt)
    st["q2nT"] = q2nT


def _s2_ffn1_x(nc, pools, W, st, ph, si):
    """ffn1 x-part; the pair shares one psum tile, sample si on partition
    half si (PE tile_position col = 64*si)."""
    x_s = st["x_s"]
    lo = 64 * si
    _mm(nc, ph[lo : lo + 64, 0:512], W.WA, x_s[:, 0:512], start=True, stop=False)
    _mm(nc, ph[lo : lo + 64, 512:736], W.WA, x_s[:, 512:736], start=True, stop=False)


def _s2_ffn1_m(nc, pools, W, st, ph, si):
    q2nT, msb = st["q2nT"], st["msb"]
    lo = 64 * si
    _mm(nc, ph[lo : lo + 64, 0:512], msb, q2nT[:, 0:512], start=False, stop=True)
    _mm(nc, ph[lo : lo + 64, 512:736], msb, q2nT[:, 512:736], start=False, stop=True)


def _s2_ffn1_act(nc, pools, W, sts, ph):
    """One pair-wide prelu extract off the stacked psum tile."""
    acts = pools["acts"]
    np_ = 64 * len(sts)
    h = acts.tile([128, 736], BF16, name="h", bufs=2)
    nc.scalar.activation(h[0:np_, 0:732], ph[0:np_, 0:732], AF.Prelu,
                         bias=W.b1[0:np_], alpha=0.01)
    for si, st in enumerate(sts):
        st["h"] = h
        st["hsi"] = si


def _s2_ffn2(nc, pools, W, st):
    acts, psum = pools["acts"], pools["psum"]
    x_s, h, si = st["x_s"], st["h"], st["hsi"]
    lo = 64 * si
    pxen = psum.tile([128, 768], F32, name="pxen", tag="ps2", bufs=2)
    _mm(nc, pxen[:, 0:512], W.w2t[lo : lo + 64, :], h[lo : lo + 64, 0:512],
        start=True, stop=True)
    _mm(nc, pxen[:, 512:732], W.w2t[lo : lo + 64, :], h[lo : lo + 64, 512:732],
        start=True, stop=True)
    xen = acts.tile([C, 768], BF16, name="xen", bufs=3)
    # residual + bias folded into the extract
    nc.vector.scalar_tensor_tensor(
        out=xen[:, 0:NTOK], in0=pxen[:, 0:NTOK], scalar=W.b2, in1=x_s[:, 0:NTOK],
        op0=ALU.add, op1=ALU.add,
    )
    st["xen"] = xen


def _s2_c1(nc, pools, W, st, grp):
    """conv1 (24x24 garbage-free windows, rows split 0-11 / 12-23) + pool."""
    acts, psum = pools["acts"], pools["psum"]
    xen, s = st["xen"], st["s"]
    pc1a = psum.tile([64, 12, 24], F32, name="pc1a", tag="ps2", bufs=2)
    pc1b = psum.tile([64, 12, 24], F32, name="pc1b", tag="ps2", bufs=2)
    for ky in range(3):
        for kx in range(3):
            tap = ky * 3 + kx
            _mm(nc, pc1a, W.wc1[:, tap, :],
                _win(xen, ky * 27 + kx, [[27, 12], [1, 24]]),
                start=(tap == 0), stop=(tap == 8))
            _mm(nc, pc1b, W.wc1[:, tap, :],
                _win(xen, (ky + 12) * 27 + kx, [[27, 12], [1, 24]]),
                start=(tap == 0), stop=(tap == 8))
    o1r = acts.tile([64, 576], BF16, name="o1r", bufs=2)
    o1rv = o1r.rearrange("p (h w) -> p h w", h=24)
    nc.scalar.activation(o1rv[:, 0:12, :], pc1a, AF.Relu, bias=W.bc1)
    nc.scalar.activation(o1rv[:, 12:24, :], pc1b, AF.Relu, bias=W.bc1)
    # pool: max over w-pairs via reduce (innermost), then h-pairs via TT max
    r1 = acts.tile([64, 24, 12], BF16, name="r1", bufs=2)
    o1rw = o1r.rearrange("p (h w2 wp) -> p h w2 wp", h=24, wp=2)
    nc.vector.tensor_reduce(r1, o1rw, axis=AX.X, op=ALU.max)
    g = s % CGRP
    o1pv = grp["o1p"][:, g, 0:144].rearrange("p (a b) -> p a b", a=12)
    nc.vector.tensor_max(o1pv, r1[:, 0:24:2, :], r1[:, 1:24:2, :])


def _emit_conv2_group(nc, pools, W, O2buf, grp, g0, gn):
    """conv2+pool for a group of gn samples (moving dim = gn*120)."""
    acts, psum = pools["acts"], pools["psum"]
    pc2 = psum.tile([128, CGRP, 10, 10], F32, name="pc2", tag="ps1", bufs=1)
    for ky in range(3):
        for kx in range(3):
            tap = ky * 3 + kx
            _mm(nc, pc2[:, 0:gn], W.wc2[:, tap, :],
                _win(grp["o1p"], ky * 12 + kx, [[148, gn], [12, 10], [1, 10]]),
                start=(tap == 0), stop=(tap == 8))
    o2r = acts.tile([128, CGRP, 100], BF16, name="o2r", bufs=2)
    o2rv = o2r.rearrange("p g (h w) -> p g h w", h=10)
    nc.scalar.activation(o2rv[:, 0:gn], pc2[:, 0:gn], AF.Relu, bias=W.bc2)
    n1 = acts.tile([128, CGRP, 25], F32, name="n1", bufs=2)
    n1v = n1.rearrange("p g (a b) -> p g a b", a=5)
    n2 = acts.tile([128, CGRP, 25], F32, name="n2", bufs=2)
    n2v = n2.rearrange("p g (a b) -> p g a b", a=5)
    nc.vector.tensor_max(
        n1v[:, 0:gn], o2rv[:, 0:gn, 0:10:2, 0:10:2], o2rv[:, 0:gn, 0:10:2, 1:10:2]
    )
    nc.vector.tensor_max(
        n2v[:, 0:gn], o2rv[:, 0:gn, 1:10:2, 0:10:2], o2rv[:, 0:gn, 1:10:2, 1:10:2]
    )
    outv = (
        O2buf[:, :, g0 : g0 + gn]
        .rearrange("p a g -> p g a")
        .rearrange("p g (a b) -> p g a b", a=5)
    )
    nc.vector.tensor_max(outv, n1v[:, 0:gn], n2v[:, 0:gn])


def _emit_fc(nc, pools, W, flags, out_dram, O2buf, ns):
    psum, fc = pools["psum"], pools["fc"]
    ones = W.ones1[0:1, 0:ns]

    po3 = psum.tile([ns, 512], F32, name="po3", tag="ps1", bufs=1)
    for p in range(25):
        _mm(nc, po3, O2buf[:, p, :], W.w1r[:, p, :],
            start=(p == 0), stop=(p == 24 and not flags["fc1_bias"]))
    if flags["fc1_bias"]:
        _mm(nc, po3, ones, W.b1row, start=False, stop=True)
    o3r = fc.tile([ns, 512], F32R, name="o3r")
    nc.scalar.activation(o3r, po3, AF.Relu)

    po3t = psum.tile([128, 4, ns], F32, name="po3t", tag="ps1", bufs=1)
    for j in range(4):
        nc.tensor.matmul(
            po3t[:, j, :].bitcast(F32R), o3r[:, 128 * j : 128 * (j + 1)],
            W.eye[0:ns, 0:ns].bitcast(F32R), is_transpose=True,
        )
    o3T = fc.tile([128, 4, ns], F32R, name="o3T")
    nc.vector.tensor_copy(o3T, po3t)

    po4 = psum.tile([ns, 512], F32, name="po4", tag="ps1", bufs=1)
    for j in range(4):
        _mm(nc, po4, o3T[:, j, :], W.wf2[:, j, :],
            start=(j == 0), stop=(j == 3 and not flags["fc2_bias"]))
    if flags["fc2_bias"]:
        _mm(nc, po4, ones, W.b2row, start=False, stop=True)
    o4r = fc.tile([ns, 512], F32R, name="o4r")
    nc.scalar.activation(o4r, po4, AF.Relu)

    po4t = psum.tile([128, 4, ns], F32, name="po4t", tag="ps1", bufs=1)
    for j in range(4):
        nc.tensor.matmul(
            po4t[:, j, :].bitcast(F32R), o4r[:, 128 * j : 128 * (j + 1)],
            W.eye[0:ns, 0:ns].bitcast(F32R), is_transpose=True,
        )
    o4T = fc.tile([128, 4, ns], F32R, name="o4T")
    nc.vector.tensor_copy(o4T, po4t)

    pcls = psum.tile([ns, 512], F32, name="pcls", tag="ps1", bufs=1)
    for j in range(4):
        _mm(nc, pcls[:, 0:16], o4T[:, j, :], W.wcls[:, j, :],
            start=(j == 0), stop=(j == 3 and not flags["cls_bias"]))
    if flags["cls_bias"]:
        _mm(nc, pcls[:, 0:16], ones, W.bcrow, start=False, stop=True)
    outsb = fc.tile([ns, 16], F32, name="outsb")
    nc.vector.tensor_copy(outsb, pcls[:, 0:16])
    nc.sync.dma_start(out=out_dram[:], in_=outsb)


def build_nc(wvals, flags, n_samples=S):
    nc = bass.Bass()
    x_dram = nc.declare_dram_parameter("x", [n_samples, C, NTOK], BF16, isOutput=False)
    out_dram = nc.declare_dram_parameter("out", [n_samples, 16], F32, isOutput=True)

    with tile.TileContext(nc) as tc:
        with (
            tc.tile_pool(name="wts", bufs=1) as wts,
            tc.tile_pool(name="acts", bufs=2) as acts,
            tc.tile_pool(name="stats", bufs=3) as stats,
            tc.tile_pool(name="fc", bufs=1) as fc,
            tc.tile_pool(name="psum", bufs=1, space="PSUM") as psum,
        ):
            pools = {"acts": acts, "stats": stats, "psum": psum, "fc": fc}
            W = _load_weights(nc, wts, wvals)
            O2buf = fc.tile([128, 25, n_samples], F32R, name="O2buf")
            grp = {}  # group-index -> {"o1p": tile}

            def tail_stages(sts):
                """Pair-i tail as 4 closures; the next pair's qkv halves are
                woven between them so psum-extract waits hide behind other
                matmul bursts (and vice versa)."""
                def t0():
                    pkv = psum.tile([128, 2, 128], F32, name="pkv", tag="ps1", bufs=1)
                    _s1_kv_pair(nc, pools, W, sts, pkv)
                    for st in sts:
                        _s1_tp(nc, pools, W, st)

                def t1():
                    ph = psum.tile([128, 768], F32, name="ph", tag="ps2", bufs=2)
                    for si, st in enumerate(sts):
                        _s2_ffn1_x(nc, pools, W, st, ph, si)
                    for si, st in enumerate(sts):
                        _s2_ffn1_m(nc, pools, W, st, ph, si)
                    _s2_ffn1_act(nc, pools, W, sts, ph)

                def t2():
                    for st in sts:
                        _s2_ffn2(nc, pools, W, st)

                def t3():
                    for st in sts:
                        _s2_c1(nc, pools, W, st, grp[st["s"] // CGRP])
                    s_last = sts[-1]["s"]
                    if s_last % CGRP == CGRP - 1 or s_last == n_samples - 1:
                        g0 = (s_last // CGRP) * CGRP
                        _emit_conv2_group(
                            nc, pools, W, O2buf, grp[g0 // CGRP], g0, s_last - g0 + 1
                        )

                return [t0, t1, t2, t3]

            prev = None
            for p0 in range(0, n_samples, 2):
                pair = [p0] + ([p0 + 1] if p0 + 1 < n_samples else [])
                sts = []
                for s in pair:
                    if s % CGRP == 0:
                        grp[s // CGRP] = {
                            "o1p": acts.tile([64, CGRP, 148], BF16, name="o1p_grp", bufs=3)
                        }
                    st = _s0_start(nc, pools, x_dram, s)
                    _s0_qkv(nc, pools, W, flags, st)
                    _s0_stats(nc, pools, W, st)
                    sts.append(st)
                if prev is not None:
                    for t in tail_stages(prev):
                        t()
                prev = sts
            for t in tail_stages(prev):
                t()
            _emit_fc(nc, pools, W, flags, out_dram, O2buf, n_samples)

    _split_waits(nc)
    return nc


_BUILD_CACHE = {}


def _make_in_maps(inputs, wvals):
    bf = mybir.dt.np(BF16)
    x = np.ascontiguousarray(np.asarray(inputs["x"], np.float32)).reshape(
        N_CORES, S, C, NTOK
    ).astype(bf)
    in_maps = []
    for c in range(N_CORES):
        m = {"x": np.ascontiguousarray(x[c])}
        m.update(wvals)
        in_maps.append(m)
    return in_maps


def kernel(**inputs):
    wvals, flags = _prep_weights(inputs)
    key = tuple(sorted(flags.items()))
    if key not in _BUILD_CACHE:
        _BUILD_CACHE[key] = build_nc(wvals, flags)
    nc = _BUILD_CACHE[key]

    in_maps = _make_in_maps(inputs, wvals)
    last_err = None
    for _attempt in range(3):
        try:
            res = run_bass_kernel_spmd(nc, in_maps, core_ids=list(range(N_CORES)))
            break
        except Exception as e:  # transient device faults: retry
            last_err = e
    else:
        raise last_err
    out = np.concatenate([res.results[c]["out"] for c in range(N_CORES)], axis=0)
    return out.astype(np.float32)


# revision 53
# speedup vs baseline: 1.0567x; 1.0567x over previous
"""Trainium2 Bass kernel for nn_Discriminator_AddDim_ESSAAttn.

Network (per sample, C=128, 27x27 spatial, N=729 tokens):
  ESSA linear attention -> concat -> 1x1-conv FFN (+residual) ->
  3x3 conv/relu/pool x2 -> 3 FC layers -> [16] logits.
Batch 256 is sharded 32-per-core across 8 NeuronCores (pure data
parallel, weights replicated).

Key algebraic folds (vs the straightforward lowering):
  - q2 row-normalisation: q2/(sum+eps) then L2-normalise == q2/||q2||_2
    (the sum cancels), so only sq4 = sum(q2^2) is needed per token.
  - attn = (v+t2) @ w_ln + b_ln is consumed ONLY by ffn1, so the whole
    attn stage folds into ffn1's weights: with WT = w_ln @ w1a,
      h = lrelu(WA^T x + WT^T (v_cm + t2_cm) + b1')
      WA = w1x + wv @ WT,  b1' = b1 + w1a^T b_ln + WT^T b_v.
  - the WT^T t2 product re-associates: WT^T (kv^T q2nT) = M^T q2nT with
    M = k2a^T (v @ WT).  v @ WT folds into the qkv weights (v-slot of
    wqkv becomes the 64-wide vW = wv @ WT slot), and M comes out of the
    same PE pass as the k2a gram (for the column norms).  The t2 psum
    stage, its 729-col extract, and the 128-col v extract all vanish.
  - the k2a column norm (invs) applies per-PARTITION on the transposed
    q2nT, so it rides the q2nT psum->sbuf extract for free.
All ESSA-chain matmuls run in bf16 (1 cyc/col on the PE, no fp32r
<256-col penalty, DVE 2x/4x fast modes on the extracts).
"""
import sys

sys.path.insert(0, "/opt/trn_rl_repo")

import numpy as np

import concourse.bass as bass
import concourse.tile as tile
from concourse import mybir
from concourse.bass_utils import run_bass_kernel_spmd

F32 = mybir.dt.float32
F32R = mybir.dt.float32r
BF16 = mybir.dt.bfloat16
AF = mybir.ActivationFunctionType
ALU = mybir.AluOpType
AX = mybir.AxisListType

N_CORES = 8
B, C, P = 256, 128, 27
NTOK = P * P          # 729
S = B // N_CORES      # 32 samples per core
NT = 6                # token tiles: 5*128 + 89
TOK_SIZES = [128, 128, 128, 128, 128, 89]
CGRP = 4              # conv2 sample-group size
QW = 320              # qkv output width: q(128) | k(128) | vW(64)


def _split_waits(nc, maxw=1):
    """walrus CoreV3 rejects instructions carrying >1 sem-wait; hoist
    extras onto preceding same-engine no-op carriers."""
    import bass_rust

    for bb in nc.m.functions[0].blocks:
        newlist = []
        for ins in bb.instructions:
            sw = ins.sync_info
            if sw and sw.on_wait and len(sw.on_wait) > maxw:
                waits = list(sw.on_wait)
                keep = waits[-maxw:]
                hoist = waits[:-maxw]
                for i in range(0, len(hoist), maxw):
                    chunk = hoist[i : i + maxw]
                    nop = bass_rust.InstNoOp(
                        name=f"{ins.name}_wsplit{i}", ins=[], outs=[]
                    )
                    nop.engine = ins.engine
                    nop.sync_info = mybir.SyncInfo(on_wait=list(chunk), on_update=[])
                    nc.register_instruction(nop, overwrite=True)
                    newlist.append(nop)
                ins.sync_info = mybir.SyncInfo(
                    on_wait=list(keep), on_update=list(sw.on_update)
                )
            newlist.append(ins)
        bb.instructions[:] = newlist


def _prep_weights(inputs):
    """Host-side weight massaging (all cheap numpy)."""
    f = lambda a: np.ascontiguousarray(np.asarray(a, np.float32))
    w_qkv = f(inputs["w_qkv"]).copy()          # [128, 384]
    b_qkv = f(inputs["b_qkv"]).copy()          # [384]
    # fold channel-mean subtraction of q and k into the weights/bias
    w_qkv[:, 0:128] -= w_qkv[:, 0:128].mean(axis=1, keepdims=True)
    w_qkv[:, 128:256] -= w_qkv[:, 128:256].mean(axis=1, keepdims=True)
    b_qkv[0:128] -= b_qkv[0:128].mean()
    b_qkv[128:256] -= b_qkv[128:256].mean()
    wv = w_qkv[:, 256:384]                     # [128, 128]
    bv = b_qkv[256:384]

    w_ln = f(inputs["w_ln"])                   # [128, 128]
    b_ln = f(inputs["b_ln"])                   # [128]
    w_ffn1 = f(inputs["w_ffn1"]).reshape(64, 256)     # [out, in]
    w1x = w_ffn1[:, 0:128].T                   # [128, 64]
    w1a = w_ffn1[:, 128:256].T                 # [128, 64]
    WT = w_ln @ w1a                            # [128, 64]
    WA = w1x + wv @ WT                         # [128, 64]
    b1p = f(inputs["b_ffn1"]) + w1a.T @ b_ln + WT.T @ bv   # [64]

    # extended qkv: q | k | vW, with vW = x^T (wv @ WT)
    wqkv_ext = np.concatenate([w_qkv[:, 0:256], wv @ WT], axis=1)  # [128, 320]
    bqkv_ext = np.concatenate([b_qkv[0:256], WT.T @ bv])           # [320]

    w2t1 = f(inputs["w_ffn2"]).reshape(128, 64).T          # [64, 128]
    # duplicated row-block: rows 64-127 serve the partition-stacked pair
    # sample (its h lives on sbuf partitions 64-127)
    w2t = np.ascontiguousarray(np.concatenate([w2t1, w2t1], axis=0))  # [128, 128]

    # conv taps -> [in_ch, 9, out_ch]
    wc1 = np.ascontiguousarray(
        f(inputs["w_c1"]).transpose(2, 3, 1, 0).reshape(9, 128, 64).transpose(1, 0, 2)
    )  # [128, 9, 64]
    wc2 = np.ascontiguousarray(
        f(inputs["w_c2"]).transpose(2, 3, 1, 0).reshape(9, 64, 128).transpose(1, 0, 2)
    )  # [64, 9, 128]

    w1r = np.ascontiguousarray(f(inputs["w_fc1"]).reshape(128, 25, 512))
    wf2 = np.ascontiguousarray(f(inputs["w_fc2"]).reshape(4, 128, 512).transpose(1, 0, 2))
    wcls = np.ascontiguousarray(f(inputs["w_cls"]).reshape(4, 128, 16).transpose(1, 0, 2))

    col = lambda a: np.ascontiguousarray(np.asarray(a, np.float32).reshape(-1, 1))
    row = lambda a: np.ascontiguousarray(np.asarray(a, np.float32).reshape(1, -1))
    w = {
        "wqkv": np.ascontiguousarray(wqkv_ext),
        "bqkv_row": row(bqkv_ext),
        "WA": np.ascontiguousarray(WA),
        "b1": col(np.concatenate([b1p, b1p])),
        "w2t": w2t,
        "b2": col(inputs["b_ffn2"]),
        "wc1": wc1,
        "bc1": col(inputs["b_c1"]),
        "wc2": wc2,
        "bc2": col(inputs["b_c2"]),
        "w1r": w1r,
        "b1row": row(inputs["b_fc1"]),
        "wf2": wf2,
        "b2row": row(inputs["b_fc2"]),
        "wcls": wcls,
        "bcrow": row(inputs["b_cls"]),
        "eye": np.eye(128, dtype=np.float32),
        "eyebf": np.eye(128, dtype=np.float32),
        "ones1": np.ones((1, S), dtype=np.float32),
    }
    flags = {
        "qkv_bias": bool(np.any(bqkv_ext)),
        "fc1_bias": bool(np.any(w["b1row"])),
        "fc2_bias": bool(np.any(w["b2row"])),
        "cls_bias": bool(np.any(w["bcrow"])),
    }
    return w, flags


class _W:
    pass


_F32_WEIGHTS = {"b1", "b2", "bc1", "bc2"}            # activation-bias operands
_BF16_WEIGHTS = {"wqkv", "WA", "w2t", "wc1", "wc2", "eyebf"}  # bf16 matmul path


def _load_weights(nc, pool, wvals):
    """Declare dram params + DMA every weight into resident SBUF tiles.
    bf16 weights are cast during a gpsimd DMA (only engine that casts)."""
    W = _W()
    for name, arr in wvals.items():
        if name in _F32_WEIGHTS:
            dt = F32
        elif name in _BF16_WEIGHTS:
            dt = BF16
        else:
            dt = F32R
        dram = nc.declare_dram_parameter(
            name, list(arr.shape), F32 if dt == BF16 else dt, isOutput=False
        )
        t = pool.tile(list(arr.shape), dt, name=f"sb_{name}")
        if dt == BF16:
            nc.gpsimd.dma_start(out=t, in_=dram[:])
        else:
            nc.sync.dma_start(out=t, in_=dram[:])
        setattr(W, name, t)
    return W


def _win(ap, offset, dims):
    """Manual sub-AP of a tile: dims = [[stride, count], ...] free dims."""
    return bass.AP(
        tensor=ap.tensor, offset=ap.offset + offset,
        ap=[list(ap.ap[0])] + [list(d) for d in dims],
    )


def _bcast(ap, nt, width):
    """[128, NT] stat tile -> broadcast AP [128, nt, width] (0-stride inner)."""
    return bass.AP(
        tensor=ap.tensor, offset=ap.offset,
        ap=[list(ap.ap[0]), [1, nt], [0, width]],
    )


def _mm(nc, out, lhsT, rhs, start=True, stop=True):
    nc.tensor.matmul(out, lhsT, rhs, start=start, stop=stop)


def _tp(nc, out, in_, eye):
    nc.tensor.matmul(out, in_, eye, is_transpose=True)


def _s0_start(nc, pools, x_dram, s):
    """Allocate the sample's tiles + launch its x DMA."""
    acts = pools["acts"]
    st = {"s": s}
    x_s = acts.tile([C, 768], BF16, name="x_s", bufs=5)
    nc.sync.dma_start(out=x_s[:, 0:NTOK], in_=x_dram[s])
    st["x_s"] = x_s
    st["q2k2"] = acts.tile([128, NT, 256], BF16, name="q2k2", bufs=3)
    st["kvsrc"] = acts.tile([128, NT, 192], BF16, name="kvsrc", bufs=4)
    return st


def _s0_qkv(nc, pools, W, flags, st):
    """qkv in two psum halves; extracts per half (ACT square + DVE vW)."""
    psum = pools["psum"]
    x_s, q2k2, kvsrc = st["x_s"], st["q2k2"], st["kvsrc"]
    for half in range(2):
        pq = psum.tile([128, 3, 512], F32, name="pq", tag="pqkv", bufs=1)
        for i in range(3):
            t = half * 3 + i
            nt = TOK_SIZES[t]
            _mm(nc, pq[0:nt, i, 0:QW], x_s[:, 128 * t : 128 * t + nt],
                W.wqkv, start=True, stop=not flags["qkv_bias"])
            if flags["qkv_bias"]:
                _mm(nc, pq[0:nt, i, 0:QW], W.ones1[0:1, 0:nt], W.bqkv_row,
                    start=False, stop=True)
        h3 = slice(3 * half, 3 * half + 3)
        nc.scalar.activation(q2k2[:, h3, :], pq[:, :, 0:256], AF.Square)
        nc.vector.tensor_copy(kvsrc[:, h3, 0:64], pq[:, :, 256:320])


def _s0_stats(nc, pools, W, st):
    """Token stats + q2n / k2a scales (needs both qkv halves)."""
    acts, stats = pools["acts"], pools["stats"]
    q2k2, kvsrc = st["q2k2"], st["kvsrc"]
    q2 = q2k2.rearrange("p t (h c) -> p t h c", h=2)[:, :, 0, :]   # [128,NT,128]
    k2 = q2k2.rearrange("p t (h c) -> p t h c", h=2)[:, :, 1, :]

    # pool queue: q4 first (deps ready immediately); the DVE k-chain runs
    # concurrently so s1kb is ready right as the pool reaches k2a.
    q4 = acts.tile([128, NT, 128], BF16, name="q4", bufs=2)
    nc.gpsimd.tensor_mul(q4, q2, q2)
    sk2 = stats.tile([128, NT], F32, name="sk2", bufs=3)
    nc.vector.reduce_sum(sk2, k2, axis=AX.X)
    # sk2 = sum_c k^2 ~ O(5) for randn inputs; the reference's +1e-7 is
    # numerically invisible, so skip the add
    s1kb = stats.tile([128, NT], BF16, name="s1kb", bufs=3)
    with nc.allow_low_precision(reason="bf16 per-token scales, ~0.4% on attn"):
        nc.vector.reciprocal(s1kb, sk2)
    nc.gpsimd.tensor_tensor(kvsrc[:, :, 64:192], k2, _bcast(s1kb, NT, 128), ALU.mult)

    sq4 = stats.tile([128, NT], F32, name="sq4", bufs=3)
    nc.vector.reduce_sum(sq4, q4, axis=AX.X)
    nq = stats.tile([128, NT], F32, name="nq", bufs=3)
    nc.scalar.activation(nq, sq4, AF.Sqrt)
    cqb = stats.tile([128, NT], BF16, name="cqb", bufs=3)
    with nc.allow_low_precision(reason="bf16 per-token scales, ~0.4% on attn"):
        nc.vector.reciprocal(cqb, nq)
    q2n = acts.tile([128, NT, 128], BF16, name="q2n", bufs=4)
    nc.gpsimd.tensor_tensor(q2n, q2, _bcast(cqb, NT, 128), ALU.mult)
    st.update(q2n=q2n, kvsrc=kvsrc)
    return st


def _s1_kv_pair(nc, pools, W, sts, pkv):
    """M = k2a^T vW and the k2a gram (column norms) in one PE pass.
    Both samples share one psum bank; gram-diag/invs chain pair-fused.
    invs lands on the msb extract (per-partition there), so q2nT stays a
    plain copy off the transpose."""
    acts, stats = pools["acts"], pools["stats"]
    npair = len(sts)
    for si, st in enumerate(sts):
        kvsrc = st["kvsrc"]
        for t in range(NT):
            nt = TOK_SIZES[t]
            _mm(nc, pkv[:, si, :], kvsrc[0:nt, t, 64:192], kvsrc[0:nt, t, :],
                start=(t == 0), stop=(t == NT - 1))
    tmpd = acts.tile([128, 2, 128], F32, name="tmpd", bufs=2)
    s2 = stats.tile([128, 2], F32, name="s2", bufs=3)
    eye_b = bass.AP(
        tensor=W.eye.tensor, offset=W.eye.offset,
        ap=[list(W.eye.ap[0]), [0, npair], [1, 128]],
    )
    nc.vector.tensor_mul(tmpd[:, 0:npair], pkv[:, 0:npair, 64:192], eye_b)
    nc.vector.reduce_sum(s2[:, 0:npair], tmpd[:, 0:npair], axis=AX.X)
    invs = stats.tile([128, 2], F32, name="invs", bufs=3)
    nc.scalar.activation(invs[:, 0:npair], s2[:, 0:npair], AF.Sqrt,
                         scale=float(NTOK))  # 27*sqrt(s2)
    nc.vector.reciprocal(invs[:, 0:npair], invs[:, 0:npair])
    for si, st in enumerate(sts):
        msb = acts.tile([128, 64], BF16, name="msb", bufs=3)
        nc.scalar.mul(msb, pkv[:, si, 0:64], invs[:, si : si + 1])
        st["msb"] = msb


def _s1_tp(nc, pools, W, st):
    """Transpose q2n to channel-major."""
    acts, psum = pools["acts"], pools["psum"]
    q2n = st["q2n"]
    pqt = psum.tile([128, 768], BF16, name="pqt", tag="ps2", bufs=2)
    for t in range(NT):
        _tp(nc, pqt[:, 128 * t : 128 * (t + 1)], q2n[:, t, :], W.eyebf)
    q2nT = acts.tile([128, 768], BF16, name="q2nT", bufs=2)
    nc.scalar.copy(q2nT, pqt)
    st["q2nT"] = q2nT


def _s2_ffn1_x(nc, pools, W, st, ph, si):
    """ffn1 x-part; the pair shares one psum tile, sample si on partition
    half si (PE tile_position col = 64*si)."""
    x_s = st["x_s"]
    lo = 64 * si
    _mm(nc, ph[lo : lo + 64, 0:512], W.WA, x_s[:, 0:512], start=True, stop=False)
    _mm(nc, ph[lo : lo + 64, 512:736], W.WA, x_s[:, 512:736], start=True, stop=False)


def _s2_ffn1_m(nc, pools, W, st, ph, si):
    q2nT, msb = st["q2nT"], st["msb"]
    lo = 64 * si
    _mm(nc, ph[lo : lo + 64, 0:512], msb, q2nT[:, 0:512], start=False, stop=True)
    _mm(nc, ph[lo : lo + 64, 512:736], msb, q2nT[:, 512:736], start=False, stop=True)


def _s2_ffn1_act(nc, pools, W, sts, ph):
    """One pair-wide prelu extract off the stacked psum tile."""
    acts = pools["acts"]
    np_ = 64 * len(sts)
    h = acts.tile([128, 736], BF16, name="h", bufs=2)
    nc.scalar.activation(h[0:np_, 0:732], ph[0:np_, 0:732], AF.Prelu,
                         bias=W.b1[0:np_], alpha=0.01)
    for si, st in enumerate(sts):
        st["h"] = h
        st["hsi"] = si


def _s2_ffn2(nc, pools, W, st):
    acts, psum = pools["acts"], pools["psum"]
    x_s, h, si = st["x_s"], st["h"], st["hsi"]
    lo = 64 * si
    pxen = psum.tile([128, 768], F32, name="pxen", tag="ps2", bufs=2)
    _mm(nc, pxen[:, 0:512], W.w2t[lo : lo + 64, :], h[lo : lo + 64, 0:512],
        start=True, stop=True)
    _mm(nc, pxen[:, 512:732], W.w2t[lo : lo + 64, :], h[lo : lo + 64, 512:732],
        start=True, stop=True)
    xen = acts.tile([C, 768], BF16, name="xen", bufs=3)
    # residual + bias folded into the extract
    nc.vector.scalar_tensor_tensor(
        out=xen[:, 0:NTOK], in0=pxen[:, 0:NTOK], scalar=W.b2, in1=x_s[:, 0:NTOK],
        op0=ALU.add, op1=ALU.add,
    )
    st["xen"] = xen


def _s2_c1(nc, pools, W, st, grp):
    """conv1 (24x24 garbage-free windows, rows split 0-11 / 12-23) + pool."""
    acts, psum = pools["acts"], pools["psum"]
    xen, s = st["xen"], st["s"]
    pc1a = psum.tile([64, 12, 24], F32, name="pc1a", tag="ps2", bufs=2)
    pc1b = psum.tile([64, 12, 24], F32, name="pc1b", tag="ps2", bufs=2)
    for ky in range(3):
        for kx in range(3):
            tap = ky * 3 + kx
            _mm(nc, pc1a, W.wc1[:, tap, :],
                _win(xen, ky * 27 + kx, [[27, 12], [1, 24]]),
                start=(tap == 0), stop=(tap == 8))
            _mm(nc, pc1b, W.wc1[:, tap, :],
                _win(xen, (ky + 12) * 27 + kx, [[27, 12], [1, 24]]),
                start=(tap == 0), stop=(tap == 8))
    o1r = acts.tile([64, 576], BF16, name="o1r", bufs=2)
    o1rv = o1r.rearrange("p (h w) -> p h w", h=24)
    nc.scalar.activation(o1rv[:, 0:12, :], pc1a, AF.Relu, bias=W.bc1)
    nc.scalar.activation(o1rv[:, 12:24, :], pc1b, AF.Relu, bias=W.bc1)
    # pool: max over w-pairs via reduce (innermost), then h-pairs via TT max
    r1 = acts.tile([64, 24, 12], BF16, name="r1", bufs=2)
    o1rw = o1r.rearrange("p (h w2 wp) -> p h w2 wp", h=24, wp=2)
    nc.vector.tensor_reduce(r1, o1rw, axis=AX.X, op=ALU.max)
    g = s % CGRP
    o1pv = grp["o1p"][:, g, 0:144].rearrange("p (a b) -> p a b", a=12)
    nc.vector.tensor_max(o1pv, r1[:, 0:24:2, :], r1[:, 1:24:2, :])


def _emit_conv2_group(nc, pools, W, O2buf, grp, g0, gn):
    """conv2+pool for a group of gn samples (moving dim = gn*120)."""
    acts, psum = pools["acts"], pools["psum"]
    pc2 = psum.tile([128, CGRP, 10, 10], F32, name="pc2", tag="ps1", bufs=1)
    for ky in range(3):
        for kx in range(3):
            tap = ky * 3 + kx
            _mm(nc, pc2[:, 0:gn], W.wc2[:, tap, :],
                _win(grp["o1p"], ky * 12 + kx, [[148, gn], [12, 10], [1, 10]]),
                start=(tap == 0), stop=(tap == 8))
    o2r = acts.tile([128, CGRP, 100], BF16, name="o2r", bufs=2)
    o2rv = o2r.rearrange("p g (h w) -> p g h w", h=10)
    nc.scalar.activation(o2rv[:, 0:gn], pc2[:, 0:gn], AF.Relu, bias=W.bc2)
    n1 = acts.tile([128, CGRP, 25], F32, name="n1", bufs=2)
    n1v = n1.rearrange("p g (a b) -> p g a b", a=5)
    n2 = acts.tile([128, CGRP, 25], F32, name="n2", bufs=2)
    n2v = n2.rearrange("p g (a b) -> p g a b", a=5)
    nc.vector.tensor_max(
        n1v[:, 0:gn], o2rv[:, 0:gn, 0:10:2, 0:10:2], o2rv[:, 0:gn, 0:10:2, 1:10:2]
    )
    nc.vector.tensor_max(
        n2v[:, 0:gn], o2rv[:, 0:gn, 1:10:2, 0:10:2], o2rv[:, 0:gn, 1:10:2, 1:10:2]
    )
    outv = (
        O2buf[:, :, g0 : g0 + gn]
        .rearrange("p a g -> p g a")
        .rearrange("p g (a b) -> p g a b", a=5)
    )
    nc.vector.tensor_max(outv, n1v[:, 0:gn], n2v[:, 0:gn])


def _emit_fc(nc, pools, W, flags, out_dram, O2buf, ns):
    psum, fc = pools["psum"], pools["fc"]
    ones = W.ones1[0:1, 0:ns]

    po3 = psum.tile([ns, 512], F32, name="po3", tag="ps1", bufs=1)
    for p in range(25):
        _mm(nc, po3, O2buf[:, p, :], W.w1r[:, p, :],
            start=(p == 0), stop=(p == 24 and not flags["fc1_bias"]))
    if flags["fc1_bias"]:
        _mm(nc, po3, ones, W.b1row, start=False, stop=True)
    o3r = fc.tile([ns, 512], F32R, name="o3r")
    nc.scalar.activation(o3r, po3, AF.Relu)

    po3t = psum.tile([128, 4, ns], F32, name="po3t", tag="ps1", bufs=1)
    for j in range(4):
        nc.tensor.matmul(
            po3t[:, j, :].bitcast(F32R), o3r[:, 128 * j : 128 * (j + 1)],
            W.eye[0:ns, 0:ns].bitcast(F32R), is_transpose=True,
        )
    o3T = fc.tile([128, 4, ns], F32R, name="o3T")
    nc.vector.tensor_copy(o3T, po3t)

    po4 = psum.tile([ns, 512], F32, name="po4", tag="ps1", bufs=1)
    for j in range(4):
        _mm(nc, po4, o3T[:, j, :], W.wf2[:, j, :],
            start=(j == 0), stop=(j == 3 and not flags["fc2_bias"]))
    if flags["fc2_bias"]:
        _mm(nc, po4, ones, W.b2row, start=False, stop=True)
    o4r = fc.tile([ns, 512], F32R, name="o4r")
    nc.scalar.activation(o4r, po4, AF.Relu)

    po4t = psum.tile([128, 4, ns], F32, name="po4t", tag="ps1", bufs=1)
    for j in range(4):
        nc.tensor.matmul(
            po4t[:, j, :].bitcast(F32R), o4r[:, 128 * j : 128 * (j + 1)],
            W.eye[0:ns, 0:ns].bitcast(F32R), is_transpose=True,
        )
    o4T = fc.tile([128, 4, ns], F32R, name="o4T")
    nc.vector.tensor_copy(o4T, po4t)

    pcls = psum.tile([ns, 512], F32, name="pcls", tag="ps1", bufs=1)
    for j in range(4):
        _mm(nc, pcls[:, 0:16], o4T[:, j, :], W.wcls[:, j, :],
            start=(j == 0), stop=(j == 3 and not flags["cls_bias"]))
    if flags["cls_bias"]:
        _mm(nc, pcls[:, 0:16], ones, W.bcrow, start=False, stop=True)
    outsb = fc.tile([ns, 16], F32, name="outsb")
    nc.vector.tensor_copy(outsb, pcls[:, 0:16])
    nc.sync.dma_start(out=out_dram[:], in_=outsb)


def build_nc(wvals, flags, n_samples=S):
    nc = bass.Bass()
    x_dram = nc.declare_dram_parameter("x", [n_samples, C, NTOK], BF16, isOutput=False)
    out_dram = nc.declare_dram_parameter("out", [n_samples, 16], F32, isOutput=True)

    with tile.TileContext(nc) as tc:
        with (
            tc.tile_pool(name="wts", bufs=1) as wts,
            tc.tile_pool(name="acts", bufs=2) as acts,
            tc.tile_pool(name="stats", bufs=3) as stats,
            tc.tile_pool(name="fc", bufs=1) as fc,
            tc.tile_pool(name="psum", bufs=1, space="PSUM") as psum,
        ):
            pools = {"acts": acts, "stats": stats, "psum": psum, "fc": fc}
            W = _load_weights(nc, wts, wvals)
            O2buf = fc.tile([128, 25, n_samples], F32R, name="O2buf")
            grp = {}  # group-index -> {"o1p": tile}

            def tail_stages(sts):
                """Pair-i tail as 4 closures; the next pair's qkv halves are
                woven between them so psum-extract waits hide behind other
                matmul bursts (and vice versa)."""
                def t0():
                    pkv = psum.tile([128, 2, 192], F32, name="pkv", tag="ps1", bufs=1)
                    _s1_kv_pair(nc, pools, W, sts, pkv)
                    for st in sts:
                        _s1_tp(nc, pools, W, st)

                def t1():
                    ph = psum.tile([128, 768], F32, name="ph", tag="ps2", bufs=2)
                    for si, st in enumerate(sts):
                        _s2_ffn1_x(nc, pools, W, st, ph, si)
                    for si, st in enumerate(sts):
                        _s2_ffn1_m(nc, pools, W, st, ph, si)
                    _s2_ffn1_act(nc, pools, W, sts, ph)

                def t2():
                    for st in sts:
                        _s2_ffn2(nc, pools, W, st)

                def t3():
                    for st in sts:
                        _s2_c1(nc, pools, W, st, grp[st["s"] // CGRP])
                    s_last = sts[-1]["s"]
                    if s_last % CGRP == CGRP - 1 or s_last == n_samples - 1:
                        g0 = (s_last // CGRP) * CGRP
                        _emit_conv2_group(
                            nc, pools, W, O2buf, grp[g0 // CGRP], g0, s_last - g0 + 1
                        )

                return [t0, t1, t2, t3]

            prev = None
            for p0 in range(0, n_samples, 2):
                pair = [p0] + ([p0 + 1] if p0 + 1 < n_samples else [])
                sts = []
                for s in pair:
                    if s % CGRP == 0:
                        grp[s // CGRP] = {
                            "o1p": acts.tile([64, CGRP, 148], BF16, name="o1p_grp", bufs=3)
                        }
                    st = _s0_start(nc, pools, x_dram, s)
                    _s0_qkv(nc, pools, W, flags, st)
                    _s0_stats(nc, pools, W, st)
                    sts.append(st)
                if prev is not None:
                    for t in tail_stages(prev):
                        t()
                prev = sts
            for t in tail_stages(prev):
                t()
            _emit_fc(nc, pools, W, flags, out_dram, O2buf, n_samples)

    _split_waits(nc)
    return nc


_BUILD_CACHE = {}


def _make_in_maps(inputs, wvals):
    bf = mybir.dt.np(BF16)
    x = np.ascontiguousarray(np.asarray(inputs["x"], np.float32)).reshape(
        N_CORES, S, C, NTOK
    ).astype(bf)
    in_maps = []
    for c in range(N_CORES):
        m = {"x": np.ascontiguousarray(x[c])}
        m.update(wvals)
        in_maps.append(m)
    return in_maps


def kernel(**inputs):
    wvals, flags = _prep_weights(inputs)
    key = tuple(sorted(flags.items()))
    if key not in _BUILD_CACHE:
        _BUILD_CACHE[key] = build_nc(wvals, flags)
    nc = _BUILD_CACHE[key]

    in_maps = _make_in_maps(inputs, wvals)
    last_err = None
    for _attempt in range(3):
        try:
            res = run_bass_kernel_spmd(nc, in_maps, core_ids=list(range(N_CORES)))
            break
        except Exception as e:  # transient device faults: retry
            last_err = e
    else:
        raise last_err
    out = np.concatenate([res.results[c]["out"] for c in range(N_CORES)], axis=0)
    return out.astype(np.float32)


# revision 54
# speedup vs baseline: 1.0774x; 1.0196x over previous
"""Trainium2 Bass kernel for nn_Discriminator_AddDim_ESSAAttn.

Network (per sample, C=128, 27x27 spatial, N=729 tokens):
  ESSA linear attention -> concat -> 1x1-conv FFN (+residual) ->
  3x3 conv/relu/pool x2 -> 3 FC layers -> [16] logits.
Batch 256 is sharded 32-per-core across 8 NeuronCores (pure data
parallel, weights replicated).

Key algebraic folds (vs the straightforward lowering):
  - q2 row-normalisation: q2/(sum+eps) then L2-normalise == q2/||q2||_2
    (the sum cancels), so only sq4 = sum(q2^2) is needed per token.
  - attn = (v+t2) @ w_ln + b_ln is consumed ONLY by ffn1, so the whole
    attn stage folds into ffn1's weights: with WT = w_ln @ w1a,
      h = lrelu(WA^T x + WT^T (v_cm + t2_cm) + b1')
      WA = w1x + wv @ WT,  b1' = b1 + w1a^T b_ln + WT^T b_v.
  - the WT^T t2 product re-associates: WT^T (kv^T q2nT) = M^T q2nT with
    M = k2a^T (v @ WT).  v @ WT folds into the qkv weights (v-slot of
    wqkv becomes the 64-wide vW = wv @ WT slot), and M comes out of the
    same PE pass as the k2a gram (for the column norms).  The t2 psum
    stage, its 729-col extract, and the 128-col v extract all vanish.
  - the k2a column norm (invs) applies per-PARTITION on the transposed
    q2nT, so it rides the q2nT psum->sbuf extract for free.
All ESSA-chain matmuls run in bf16 (1 cyc/col on the PE, no fp32r
<256-col penalty, DVE 2x/4x fast modes on the extracts).
"""
import sys

sys.path.insert(0, "/opt/trn_rl_repo")

import numpy as np

import concourse.bass as bass
import concourse.tile as tile
from concourse import mybir
from concourse.bass_utils import run_bass_kernel_spmd

F32 = mybir.dt.float32
F32R = mybir.dt.float32r
BF16 = mybir.dt.bfloat16
AF = mybir.ActivationFunctionType
ALU = mybir.AluOpType
AX = mybir.AxisListType

N_CORES = 8
B, C, P = 256, 128, 27
NTOK = P * P          # 729
S = B // N_CORES      # 32 samples per core
NT = 6                # token tiles: 5*128 + 89
TOK_SIZES = [128, 128, 128, 128, 128, 89]
CGRP = 4              # conv2 sample-group size
QW = 320              # qkv output width: q(128) | k(128) | vW(64)


def _split_waits(nc, maxw=1):
    """walrus CoreV3 rejects instructions carrying >1 sem-wait; hoist
    extras onto preceding same-engine no-op carriers."""
    import bass_rust

    for bb in nc.m.functions[0].blocks:
        newlist = []
        for ins in bb.instructions:
            sw = ins.sync_info
            if sw and sw.on_wait and len(sw.on_wait) > maxw:
                waits = list(sw.on_wait)
                keep = waits[-maxw:]
                hoist = waits[:-maxw]
                for i in range(0, len(hoist), maxw):
                    chunk = hoist[i : i + maxw]
                    nop = bass_rust.InstNoOp(
                        name=f"{ins.name}_wsplit{i}", ins=[], outs=[]
                    )
                    nop.engine = ins.engine
                    nop.sync_info = mybir.SyncInfo(on_wait=list(chunk), on_update=[])
                    nc.register_instruction(nop, overwrite=True)
                    newlist.append(nop)
                ins.sync_info = mybir.SyncInfo(
                    on_wait=list(keep), on_update=list(sw.on_update)
                )
            newlist.append(ins)
        bb.instructions[:] = newlist


def _prep_weights(inputs):
    """Host-side weight massaging (all cheap numpy)."""
    f = lambda a: np.ascontiguousarray(np.asarray(a, np.float32))
    w_qkv = f(inputs["w_qkv"]).copy()          # [128, 384]
    b_qkv = f(inputs["b_qkv"]).copy()          # [384]
    # fold channel-mean subtraction of q and k into the weights/bias
    w_qkv[:, 0:128] -= w_qkv[:, 0:128].mean(axis=1, keepdims=True)
    w_qkv[:, 128:256] -= w_qkv[:, 128:256].mean(axis=1, keepdims=True)
    b_qkv[0:128] -= b_qkv[0:128].mean()
    b_qkv[128:256] -= b_qkv[128:256].mean()
    wv = w_qkv[:, 256:384]                     # [128, 128]
    bv = b_qkv[256:384]

    w_ln = f(inputs["w_ln"])                   # [128, 128]
    b_ln = f(inputs["b_ln"])                   # [128]
    w_ffn1 = f(inputs["w_ffn1"]).reshape(64, 256)     # [out, in]
    w1x = w_ffn1[:, 0:128].T                   # [128, 64]
    w1a = w_ffn1[:, 128:256].T                 # [128, 64]
    WT = w_ln @ w1a                            # [128, 64]
    WA = w1x + wv @ WT                         # [128, 64]
    b1p = f(inputs["b_ffn1"]) + w1a.T @ b_ln + WT.T @ bv   # [64]

    # extended qkv: q | k | vW, with vW = x^T (wv @ WT)
    wqkv_ext = np.concatenate([w_qkv[:, 0:256], wv @ WT], axis=1)  # [128, 320]
    bqkv_ext = np.concatenate([b_qkv[0:256], WT.T @ bv])           # [320]

    w2t1 = f(inputs["w_ffn2"]).reshape(128, 64).T          # [64, 128]
    # duplicated row-block: rows 64-127 serve the partition-stacked pair
    # sample (its h lives on sbuf partitions 64-127)
    w2t = np.ascontiguousarray(np.concatenate([w2t1, w2t1], axis=0))  # [128, 128]

    # conv taps -> [in_ch, 9, out_ch]
    wc1 = np.ascontiguousarray(
        f(inputs["w_c1"]).transpose(2, 3, 1, 0).reshape(9, 128, 64).transpose(1, 0, 2)
    )  # [128, 9, 64]
    wc2 = np.ascontiguousarray(
        f(inputs["w_c2"]).transpose(2, 3, 1, 0).reshape(9, 64, 128).transpose(1, 0, 2)
    )  # [64, 9, 128]

    w1r = np.ascontiguousarray(f(inputs["w_fc1"]).reshape(128, 25, 512))
    wf2 = np.ascontiguousarray(f(inputs["w_fc2"]).reshape(4, 128, 512).transpose(1, 0, 2))
    wcls = np.ascontiguousarray(f(inputs["w_cls"]).reshape(4, 128, 16).transpose(1, 0, 2))

    col = lambda a: np.ascontiguousarray(np.asarray(a, np.float32).reshape(-1, 1))
    row = lambda a: np.ascontiguousarray(np.asarray(a, np.float32).reshape(1, -1))
    w = {
        "wqkv": np.ascontiguousarray(wqkv_ext),
        "bqkv_row": row(bqkv_ext),
        "WA": np.ascontiguousarray(WA),
        "b1": col(np.concatenate([b1p, b1p])),
        "w2t": w2t,
        "b2": col(inputs["b_ffn2"]),
        "wc1": wc1,
        "bc1": col(inputs["b_c1"]),
        "wc2": wc2,
        "bc2": col(inputs["b_c2"]),
        "w1r": w1r,
        "b1row": row(inputs["b_fc1"]),
        "wf2": wf2,
        "b2row": row(inputs["b_fc2"]),
        "wcls": wcls,
        "bcrow": row(inputs["b_cls"]),
        "eye": np.eye(128, dtype=np.float32),
        "eyebf": np.eye(128, dtype=np.float32),
        "ones1": np.ones((1, S), dtype=np.float32),
    }
    flags = {
        "qkv_bias": bool(np.any(bqkv_ext)),
        "fc1_bias": bool(np.any(w["b1row"])),
        "fc2_bias": bool(np.any(w["b2row"])),
        "cls_bias": bool(np.any(w["bcrow"])),
    }
    return w, flags


class _W:
    pass


_F32_WEIGHTS = {"b1", "b2", "bc1", "bc2"}            # activation-bias operands
_BF16_WEIGHTS = {"wqkv", "WA", "w2t", "wc1", "wc2", "eyebf"}  # bf16 matmul path


def _load_weights(nc, pool, wvals):
    """Declare dram params + DMA every weight into resident SBUF tiles.
    bf16 weights are cast during a gpsimd DMA (only engine that casts)."""
    W = _W()
    for name, arr in wvals.items():
        if name in _F32_WEIGHTS:
            dt = F32
        elif name in _BF16_WEIGHTS:
            dt = BF16
        else:
            dt = F32R
        dram = nc.declare_dram_parameter(
            name, list(arr.shape), F32 if dt == BF16 else dt, isOutput=False
        )
        t = pool.tile(list(arr.shape), dt, name=f"sb_{name}")
        if dt == BF16:
            nc.gpsimd.dma_start(out=t, in_=dram[:])
        else:
            nc.sync.dma_start(out=t, in_=dram[:])
        setattr(W, name, t)
    return W


def _win(ap, offset, dims):
    """Manual sub-AP of a tile: dims = [[stride, count], ...] free dims."""
    return bass.AP(
        tensor=ap.tensor, offset=ap.offset + offset,
        ap=[list(ap.ap[0])] + [list(d) for d in dims],
    )


def _bcast(ap, nt, width):
    """[128, NT] stat tile -> broadcast AP [128, nt, width] (0-stride inner)."""
    return bass.AP(
        tensor=ap.tensor, offset=ap.offset,
        ap=[list(ap.ap[0]), [1, nt], [0, width]],
    )


def _mm(nc, out, lhsT, rhs, start=True, stop=True):
    nc.tensor.matmul(out, lhsT, rhs, start=start, stop=stop)


def _tp(nc, out, in_, eye):
    nc.tensor.matmul(out, in_, eye, is_transpose=True)


def _s0_start(nc, pools, x_dram, s):
    """Allocate the sample's tiles + launch its x DMA."""
    acts = pools["acts"]
    st = {"s": s}
    x_s = acts.tile([C, 768], BF16, name="x_s", bufs=5)
    nc.sync.dma_start(out=x_s[:, 0:NTOK], in_=x_dram[s])
    st["x_s"] = x_s
    st["q2k2"] = acts.tile([128, NT, 256], BF16, name="q2k2", bufs=3)
    st["kvsrc"] = acts.tile([128, NT, 192], BF16, name="kvsrc", bufs=4)
    return st


def _s0_qkv(nc, pools, W, flags, st):
    """qkv in two psum halves; extracts per half (ACT square + DVE vW)."""
    psum = pools["psum"]
    x_s, q2k2, kvsrc = st["x_s"], st["q2k2"], st["kvsrc"]
    for half in range(2):
        pq = psum.tile([128, 3, 512], F32, name="pq", tag="pqkv", bufs=1)
        for i in range(3):
            t = half * 3 + i
            nt = TOK_SIZES[t]
            _mm(nc, pq[0:nt, i, 0:QW], x_s[:, 128 * t : 128 * t + nt],
                W.wqkv, start=True, stop=not flags["qkv_bias"])
            if flags["qkv_bias"]:
                _mm(nc, pq[0:nt, i, 0:QW], W.ones1[0:1, 0:nt], W.bqkv_row,
                    start=False, stop=True)
        h3 = slice(3 * half, 3 * half + 3)
        nc.scalar.activation(q2k2[:, h3, :], pq[:, :, 0:256], AF.Square)
        nc.vector.tensor_copy(kvsrc[:, h3, 0:64], pq[:, :, 256:320])


def _s0_stats(nc, pools, W, st):
    """Token stats + q2n / k2a scales (needs both qkv halves)."""
    acts, stats = pools["acts"], pools["stats"]
    q2k2, kvsrc = st["q2k2"], st["kvsrc"]
    q2 = q2k2.rearrange("p t (h c) -> p t h c", h=2)[:, :, 0, :]   # [128,NT,128]
    k2 = q2k2.rearrange("p t (h c) -> p t h c", h=2)[:, :, 1, :]

    # DVE k-chain first so k2a (pool's first op) unblocks quickly; q4 runs
    # on DVE afterwards, in parallel with the pool's k2a.
    sk2 = stats.tile([128, NT], F32, name="sk2", bufs=3)
    nc.vector.reduce_sum(sk2, k2, axis=AX.X)
    # sk2 = sum_c k^2 ~ O(5) for randn inputs; the reference's +1e-7 is
    # numerically invisible, so skip the add
    s1kb = stats.tile([128, NT], BF16, name="s1kb", bufs=3)
    with nc.allow_low_precision(reason="bf16 per-token scales, ~0.4% on attn"):
        nc.vector.reciprocal(s1kb, sk2)
    nc.gpsimd.tensor_tensor(kvsrc[:, :, 64:192], k2, _bcast(s1kb, NT, 128), ALU.mult)
    q4 = acts.tile([128, NT, 128], BF16, name="q4", bufs=2)
    nc.vector.tensor_mul(q4, q2, q2)

    sq4 = stats.tile([128, NT], F32, name="sq4", bufs=3)
    nc.vector.reduce_sum(sq4, q4, axis=AX.X)
    nq = stats.tile([128, NT], F32, name="nq", bufs=3)
    nc.scalar.activation(nq, sq4, AF.Sqrt)
    cqb = stats.tile([128, NT], BF16, name="cqb", bufs=3)
    with nc.allow_low_precision(reason="bf16 per-token scales, ~0.4% on attn"):
        nc.vector.reciprocal(cqb, nq)
    q2n = acts.tile([128, NT, 128], BF16, name="q2n", bufs=4)
    nc.gpsimd.tensor_tensor(q2n, q2, _bcast(cqb, NT, 128), ALU.mult)
    st.update(q2n=q2n, kvsrc=kvsrc)
    return st


def _s1_kv_pair(nc, pools, W, sts, pkv):
    """M = k2a^T vW and the k2a gram (column norms) in one PE pass.
    Both samples share one psum bank; gram-diag/invs chain pair-fused.
    invs lands on the msb extract (per-partition there), so q2nT stays a
    plain copy off the transpose."""
    acts, stats = pools["acts"], pools["stats"]
    npair = len(sts)
    for si, st in enumerate(sts):
        kvsrc = st["kvsrc"]
        for t in range(NT):
            nt = TOK_SIZES[t]
            _mm(nc, pkv[:, si, :], kvsrc[0:nt, t, 64:192], kvsrc[0:nt, t, :],
                start=(t == 0), stop=(t == NT - 1))
    tmpd = acts.tile([128, 2, 128], F32, name="tmpd", bufs=2)
    s2 = stats.tile([128, 2], F32, name="s2", bufs=3)
    eye_b = bass.AP(
        tensor=W.eye.tensor, offset=W.eye.offset,
        ap=[list(W.eye.ap[0]), [0, npair], [1, 128]],
    )
    nc.vector.tensor_mul(tmpd[:, 0:npair], pkv[:, 0:npair, 64:192], eye_b)
    nc.vector.reduce_sum(s2[:, 0:npair], tmpd[:, 0:npair], axis=AX.X)
    invs = stats.tile([128, 2], F32, name="invs", bufs=3)
    nc.scalar.activation(invs[:, 0:npair], s2[:, 0:npair], AF.Sqrt,
                         scale=float(NTOK))  # 27*sqrt(s2)
    nc.vector.reciprocal(invs[:, 0:npair], invs[:, 0:npair])
    for si, st in enumerate(sts):
        msb = acts.tile([128, 64], BF16, name="msb", bufs=3)
        nc.scalar.mul(msb, pkv[:, si, 0:64], invs[:, si : si + 1])
        st["msb"] = msb


def _s1_tp(nc, pools, W, st):
    """Transpose q2n to channel-major."""
    acts, psum = pools["acts"], pools["psum"]
    q2n = st["q2n"]
    pqt = psum.tile([128, 768], BF16, name="pqt", tag="ps2", bufs=2)
    for t in range(NT):
        _tp(nc, pqt[:, 128 * t : 128 * (t + 1)], q2n[:, t, :], W.eyebf)
    q2nT = acts.tile([128, 768], BF16, name="q2nT", bufs=2)
    nc.scalar.copy(q2nT, pqt)
    st["q2nT"] = q2nT


def _s2_ffn1_x(nc, pools, W, st, ph, si):
    """ffn1 x-part; the pair shares one psum tile, sample si on partition
    half si (PE tile_position col = 64*si)."""
    x_s = st["x_s"]
    lo = 64 * si
    _mm(nc, ph[lo : lo + 64, 0:512], W.WA, x_s[:, 0:512], start=True, stop=False)
    _mm(nc, ph[lo : lo + 64, 512:736], W.WA, x_s[:, 512:736], start=True, stop=False)


def _s2_ffn1_m(nc, pools, W, st, ph, si):
    q2nT, msb = st["q2nT"], st["msb"]
    lo = 64 * si
    _mm(nc, ph[lo : lo + 64, 0:512], msb, q2nT[:, 0:512], start=False, stop=True)
    _mm(nc, ph[lo : lo + 64, 512:736], msb, q2nT[:, 512:736], start=False, stop=True)


def _s2_ffn1_act(nc, pools, W, sts, ph):
    """One pair-wide prelu extract off the stacked psum tile."""
    acts = pools["acts"]
    np_ = 64 * len(sts)
    h = acts.tile([128, 736], BF16, name="h", bufs=2)
    nc.scalar.activation(h[0:np_, 0:732], ph[0:np_, 0:732], AF.Prelu,
                         bias=W.b1[0:np_], alpha=0.01)
    for si, st in enumerate(sts):
        st["h"] = h
        st["hsi"] = si


def _s2_ffn2(nc, pools, W, st):
    acts, psum = pools["acts"], pools["psum"]
    x_s, h, si = st["x_s"], st["h"], st["hsi"]
    lo = 64 * si
    pxen = psum.tile([128, 768], F32, name="pxen", tag="ps2", bufs=2)
    _mm(nc, pxen[:, 0:512], W.w2t[lo : lo + 64, :], h[lo : lo + 64, 0:512],
        start=True, stop=True)
    _mm(nc, pxen[:, 512:732], W.w2t[lo : lo + 64, :], h[lo : lo + 64, 512:732],
        start=True, stop=True)
    xen = acts.tile([C, 768], BF16, name="xen", bufs=3)
    # residual + bias folded into the extract
    nc.vector.scalar_tensor_tensor(
        out=xen[:, 0:NTOK], in0=pxen[:, 0:NTOK], scalar=W.b2, in1=x_s[:, 0:NTOK],
        op0=ALU.add, op1=ALU.add,
    )
    st["xen"] = xen


def _s2_c1(nc, pools, W, st, grp):
    """conv1 (24x24 garbage-free windows, rows split 0-11 / 12-23) + pool."""
    acts, psum = pools["acts"], pools["psum"]
    xen, s = st["xen"], st["s"]
    pc1a = psum.tile([64, 12, 24], F32, name="pc1a", tag="ps2", bufs=2)
    pc1b = psum.tile([64, 12, 24], F32, name="pc1b", tag="ps2", bufs=2)
    for ky in range(3):
        for kx in range(3):
            tap = ky * 3 + kx
            _mm(nc, pc1a, W.wc1[:, tap, :],
                _win(xen, ky * 27 + kx, [[27, 12], [1, 24]]),
                start=(tap == 0), stop=(tap == 8))
            _mm(nc, pc1b, W.wc1[:, tap, :],
                _win(xen, (ky + 12) * 27 + kx, [[27, 12], [1, 24]]),
                start=(tap == 0), stop=(tap == 8))
    o1r = acts.tile([64, 576], BF16, name="o1r", bufs=2)
    o1rv = o1r.rearrange("p (h w) -> p h w", h=24)
    nc.scalar.activation(o1rv[:, 0:12, :], pc1a, AF.Relu, bias=W.bc1)
    nc.scalar.activation(o1rv[:, 12:24, :], pc1b, AF.Relu, bias=W.bc1)
    # pool: max over w-pairs via reduce (innermost), then h-pairs via TT max
    r1 = acts.tile([64, 24, 12], BF16, name="r1", bufs=2)
    o1rw = o1r.rearrange("p (h w2 wp) -> p h w2 wp", h=24, wp=2)
    nc.vector.tensor_reduce(r1, o1rw, axis=AX.X, op=ALU.max)
    g = s % CGRP
    o1pv = grp["o1p"][:, g, 0:144].rearrange("p (a b) -> p a b", a=12)
    nc.vector.tensor_max(o1pv, r1[:, 0:24:2, :], r1[:, 1:24:2, :])


def _emit_conv2_group(nc, pools, W, O2buf, grp, g0, gn):
    """conv2+pool for a group of gn samples (moving dim = gn*120)."""
    acts, psum = pools["acts"], pools["psum"]
    pc2 = psum.tile([128, CGRP, 10, 10], F32, name="pc2", tag="ps1", bufs=1)
    for ky in range(3):
        for kx in range(3):
            tap = ky * 3 + kx
            _mm(nc, pc2[:, 0:gn], W.wc2[:, tap, :],
                _win(grp["o1p"], ky * 12 + kx, [[148, gn], [12, 10], [1, 10]]),
                start=(tap == 0), stop=(tap == 8))
    o2r = acts.tile([128, CGRP, 100], BF16, name="o2r", bufs=2)
    o2rv = o2r.rearrange("p g (h w) -> p g h w", h=10)
    nc.scalar.activation(o2rv[:, 0:gn], pc2[:, 0:gn], AF.Relu, bias=W.bc2)
    n1 = acts.tile([128, CGRP, 25], F32, name="n1", bufs=2)
    n1v = n1.rearrange("p g (a b) -> p g a b", a=5)
    n2 = acts.tile([128, CGRP, 25], F32, name="n2", bufs=2)
    n2v = n2.rearrange("p g (a b) -> p g a b", a=5)
    nc.vector.tensor_max(
        n1v[:, 0:gn], o2rv[:, 0:gn, 0:10:2, 0:10:2], o2rv[:, 0:gn, 0:10:2, 1:10:2]
    )
    nc.vector.tensor_max(
        n2v[:, 0:gn], o2rv[:, 0:gn, 1:10:2, 0:10:2], o2rv[:, 0:gn, 1:10:2, 1:10:2]
    )
    outv = (
        O2buf[:, :, g0 : g0 + gn]
        .rearrange("p a g -> p g a")
        .rearrange("p g (a b) -> p g a b", a=5)
    )
    nc.vector.tensor_max(outv, n1v[:, 0:gn], n2v[:, 0:gn])


def _emit_fc(nc, pools, W, flags, out_dram, O2buf, ns):
    psum, fc = pools["psum"], pools["fc"]
    ones = W.ones1[0:1, 0:ns]

    po3 = psum.tile([ns, 512], F32, name="po3", tag="ps1", bufs=1)
    for p in range(25):
        _mm(nc, po3, O2buf[:, p, :], W.w1r[:, p, :],
            start=(p == 0), stop=(p == 24 and not flags["fc1_bias"]))
    if flags["fc1_bias"]:
        _mm(nc, po3, ones, W.b1row, start=False, stop=True)
    o3r = fc.tile([ns, 512], F32R, name="o3r")
    nc.scalar.activation(o3r, po3, AF.Relu)

    po3t = psum.tile([128, 4, ns], F32, name="po3t", tag="ps1", bufs=1)
    for j in range(4):
        nc.tensor.matmul(
            po3t[:, j, :].bitcast(F32R), o3r[:, 128 * j : 128 * (j + 1)],
            W.eye[0:ns, 0:ns].bitcast(F32R), is_transpose=True,
        )
    o3T = fc.tile([128, 4, ns], F32R, name="o3T")
    nc.vector.tensor_copy(o3T, po3t)

    po4 = psum.tile([ns, 512], F32, name="po4", tag="ps1", bufs=1)
    for j in range(4):
        _mm(nc, po4, o3T[:, j, :], W.wf2[:, j, :],
            start=(j == 0), stop=(j == 3 and not flags["fc2_bias"]))
    if flags["fc2_bias"]:
        _mm(nc, po4, ones, W.b2row, start=False, stop=True)
    o4r = fc.tile([ns, 512], F32R, name="o4r")
    nc.scalar.activation(o4r, po4, AF.Relu)

    po4t = psum.tile([128, 4, ns], F32, name="po4t", tag="ps1", bufs=1)
    for j in range(4):
        nc.tensor.matmul(
            po4t[:, j, :].bitcast(F32R), o4r[:, 128 * j : 128 * (j + 1)],
            W.eye[0:ns, 0:ns].bitcast(F32R), is_transpose=True,
        )
    o4T = fc.tile([128, 4, ns], F32R, name="o4T")
    nc.vector.tensor_copy(o4T, po4t)

    pcls = psum.tile([ns, 512], F32, name="pcls", tag="ps1", bufs=1)
    for j in range(4):
        _mm(nc, pcls[:, 0:16], o4T[:, j, :], W.wcls[:, j, :],
            start=(j == 0), stop=(j == 3 and not flags["cls_bias"]))
    if flags["cls_bias"]:
        _mm(nc, pcls[:, 0:16], ones, W.bcrow, start=False, stop=True)
    outsb = fc.tile([ns, 16], F32, name="outsb")
    nc.vector.tensor_copy(outsb, pcls[:, 0:16])
    nc.sync.dma_start(out=out_dram[:], in_=outsb)


def build_nc(wvals, flags, n_samples=S):
    nc = bass.Bass()
    x_dram = nc.declare_dram_parameter("x", [n_samples, C, NTOK], BF16, isOutput=False)
    out_dram = nc.declare_dram_parameter("out", [n_samples, 16], F32, isOutput=True)

    with tile.TileContext(nc) as tc:
        with (
            tc.tile_pool(name="wts", bufs=1) as wts,
            tc.tile_pool(name="acts", bufs=2) as acts,
            tc.tile_pool(name="stats", bufs=3) as stats,
            tc.tile_pool(name="fc", bufs=1) as fc,
            tc.tile_pool(name="psum", bufs=1, space="PSUM") as psum,
        ):
            pools = {"acts": acts, "stats": stats, "psum": psum, "fc": fc}
            W = _load_weights(nc, wts, wvals)
            O2buf = fc.tile([128, 25, n_samples], F32R, name="O2buf")
            grp = {}  # group-index -> {"o1p": tile}

            def tail_stages(sts):
                """Pair-i tail as 4 closures; the next pair's qkv halves are
                woven between them so psum-extract waits hide behind other
                matmul bursts (and vice versa)."""
                def t0():
                    pkv = psum.tile([128, 2, 192], F32, name="pkv", tag="ps1", bufs=1)
                    _s1_kv_pair(nc, pools, W, sts, pkv)
                    for st in sts:
                        _s1_tp(nc, pools, W, st)

                def t1():
                    ph = psum.tile([128, 768], F32, name="ph", tag="ps2", bufs=2)
                    for si, st in enumerate(sts):
                        _s2_ffn1_x(nc, pools, W, st, ph, si)
                    for si, st in enumerate(sts):
                        _s2_ffn1_m(nc, pools, W, st, ph, si)
                    _s2_ffn1_act(nc, pools, W, sts, ph)

                def t2():
                    for st in sts:
                        _s2_ffn2(nc, pools, W, st)

                def t3():
                    for st in sts:
                        _s2_c1(nc, pools, W, st, grp[st["s"] // CGRP])
                    s_last = sts[-1]["s"]
                    if s_last % CGRP == CGRP - 1 or s_last == n_samples - 1:
                        g0 = (s_last // CGRP) * CGRP
                        _emit_conv2_group(
                            nc, pools, W, O2buf, grp[g0 // CGRP], g0, s_last - g0 + 1
                        )

                return [t0, t1, t2, t3]

            prev = None
            for p0 in range(0, n_samples, 2):
                pair = [p0] + ([p0 + 1] if p0 + 1 < n_samples else [])
                sts = []
                for s in pair:
                    if s % CGRP == 0:
                        grp[s // CGRP] = {
                            "o1p": acts.tile([64, CGRP, 148], BF16, name="o1p_grp", bufs=3)
                        }
                    st = _s0_start(nc, pools, x_dram, s)
                    _s0_qkv(nc, pools, W, flags, st)
                    _s0_stats(nc, pools, W, st)
                    sts.append(st)
                if prev is not None:
                    for t in tail_stages(prev):
                        t()
                prev = sts
            for t in tail_stages(prev):
                t()
            _emit_fc(nc, pools, W, flags, out_dram, O2buf, n_samples)

    _split_waits(nc)
    return nc


_BUILD_CACHE = {}


def _make_in_maps(inputs, wvals):
    bf = mybir.dt.np(BF16)
    x = np.ascontiguousarray(np.asarray(inputs["x"], np.float32)).reshape(
        N_CORES, S, C, NTOK
    ).astype(bf)
    in_maps = []
    for c in range(N_CORES):
        m = {"x": np.ascontiguousarray(x[c])}
        m.update(wvals)
        in_maps.append(m)
    return in_maps


def kernel(**inputs):
    wvals, flags = _prep_weights(inputs)
    key = tuple(sorted(flags.items()))
    if key not in _BUILD_CACHE:
        _BUILD_CACHE[key] = build_nc(wvals, flags)
    nc = _BUILD_CACHE[key]

    in_maps = _make_in_maps(inputs, wvals)
    last_err = None
    for _attempt in range(3):
        try:
            res = run_bass_kernel_spmd(nc, in_maps, core_ids=list(range(N_CORES)))
            break
        except Exception as e:  # transient device faults: retry
            last_err = e
    else:
        raise last_err
    out = np.concatenate([res.results[c]["out"] for c in range(N_CORES)], axis=0)
    return out.astype(np.float32)


# revision 55
# speedup vs baseline: 1.1356x; 1.0541x over previous
"""Trainium2 Bass kernel for nn_Discriminator_AddDim_ESSAAttn.

Network (per sample, C=128, 27x27 spatial, N=729 tokens):
  ESSA linear attention -> concat -> 1x1-conv FFN (+residual) ->
  3x3 conv/relu/pool x2 -> 3 FC layers -> [16] logits.
Batch 256 is sharded 32-per-core across 8 NeuronCores (pure data
parallel, weights replicated).

Key algebraic folds (vs the straightforward lowering):
  - q2 row-normalisation: q2/(sum+eps) then L2-normalise == q2/||q2||_2
    (the sum cancels), so only sq4 = sum(q2^2) is needed per token.
  - attn = (v+t2) @ w_ln + b_ln is consumed ONLY by ffn1, so the whole
    attn stage folds into ffn1's weights: with WT = w_ln @ w1a,
      h = lrelu(WA^T x + WT^T (v_cm + t2_cm) + b1')
      WA = w1x + wv @ WT,  b1' = b1 + w1a^T b_ln + WT^T b_v.
  - the WT^T t2 product re-associates: WT^T (kv^T q2nT) = M^T q2nT with
    M = k2a^T (v @ WT).  v @ WT folds into the qkv weights (v-slot of
    wqkv becomes the 64-wide vW = wv @ WT slot), and M comes out of the
    same PE pass as the k2a gram (for the column norms).  The t2 psum
    stage, its 729-col extract, and the 128-col v extract all vanish.
  - the k2a column norm (invs) applies per-PARTITION on the transposed
    q2nT, so it rides the q2nT psum->sbuf extract for free.
All ESSA-chain matmuls run in bf16 (1 cyc/col on the PE, no fp32r
<256-col penalty, DVE 2x/4x fast modes on the extracts).
"""
import sys

sys.path.insert(0, "/opt/trn_rl_repo")

import numpy as np

import concourse.bass as bass
import concourse.tile as tile
from concourse import mybir
from concourse.bass_utils import run_bass_kernel_spmd

F32 = mybir.dt.float32
F32R = mybir.dt.float32r
BF16 = mybir.dt.bfloat16
AF = mybir.ActivationFunctionType
ALU = mybir.AluOpType
AX = mybir.AxisListType

N_CORES = 8
B, C, P = 256, 128, 27
NTOK = P * P          # 729
S = B // N_CORES      # 32 samples per core
NT = 6                # token tiles: 5*128 + 89
TOK_SIZES = [128, 128, 128, 128, 128, 89]
CGRP = 4              # conv2 sample-group size
QW = 320              # qkv output width: q(128) | k(128) | vW(64)


def _split_waits(nc, maxw=1):
    """walrus CoreV3 rejects instructions carrying >1 sem-wait; hoist
    extras onto preceding same-engine no-op carriers."""
    import bass_rust

    for bb in nc.m.functions[0].blocks:
        newlist = []
        for ins in bb.instructions:
            sw = ins.sync_info
            if sw and sw.on_wait and len(sw.on_wait) > maxw:
                waits = list(sw.on_wait)
                keep = waits[-maxw:]
                hoist = waits[:-maxw]
                for i in range(0, len(hoist), maxw):
                    chunk = hoist[i : i + maxw]
                    nop = bass_rust.InstNoOp(
                        name=f"{ins.name}_wsplit{i}", ins=[], outs=[]
                    )
                    nop.engine = ins.engine
                    nop.sync_info = mybir.SyncInfo(on_wait=list(chunk), on_update=[])
                    nc.register_instruction(nop, overwrite=True)
                    newlist.append(nop)
                ins.sync_info = mybir.SyncInfo(
                    on_wait=list(keep), on_update=list(sw.on_update)
                )
            newlist.append(ins)
        bb.instructions[:] = newlist


def _prep_weights(inputs):
    """Host-side weight massaging (all cheap numpy)."""
    f = lambda a: np.ascontiguousarray(np.asarray(a, np.float32))
    w_qkv = f(inputs["w_qkv"]).copy()          # [128, 384]
    b_qkv = f(inputs["b_qkv"]).copy()          # [384]
    # fold channel-mean subtraction of q and k into the weights/bias
    w_qkv[:, 0:128] -= w_qkv[:, 0:128].mean(axis=1, keepdims=True)
    w_qkv[:, 128:256] -= w_qkv[:, 128:256].mean(axis=1, keepdims=True)
    b_qkv[0:128] -= b_qkv[0:128].mean()
    b_qkv[128:256] -= b_qkv[128:256].mean()
    wv = w_qkv[:, 256:384]                     # [128, 128]
    bv = b_qkv[256:384]

    w_ln = f(inputs["w_ln"])                   # [128, 128]
    b_ln = f(inputs["b_ln"])                   # [128]
    w_ffn1 = f(inputs["w_ffn1"]).reshape(64, 256)     # [out, in]
    w1x = w_ffn1[:, 0:128].T                   # [128, 64]
    w1a = w_ffn1[:, 128:256].T                 # [128, 64]
    WT = w_ln @ w1a                            # [128, 64]
    WA = w1x + wv @ WT                         # [128, 64]
    b1p = f(inputs["b_ffn1"]) + w1a.T @ b_ln + WT.T @ bv   # [64]

    # extended qkv: q | k | vW, with vW = x^T (wv @ WT)
    wqkv_ext = np.concatenate([w_qkv[:, 0:256], wv @ WT], axis=1)  # [128, 320]
    bqkv_ext = np.concatenate([b_qkv[0:256], WT.T @ bv])           # [320]

    w2t1 = f(inputs["w_ffn2"]).reshape(128, 64).T          # [64, 128]
    # duplicated row-block: rows 64-127 serve the partition-stacked pair
    # sample (its h lives on sbuf partitions 64-127)
    w2t = np.ascontiguousarray(np.concatenate([w2t1, w2t1], axis=0))  # [128, 128]

    # conv taps -> [in_ch, 9, out_ch]
    wc1 = np.ascontiguousarray(
        f(inputs["w_c1"]).transpose(2, 3, 1, 0).reshape(9, 128, 64).transpose(1, 0, 2)
    )  # [128, 9, 64]
    wc2 = np.ascontiguousarray(
        f(inputs["w_c2"]).transpose(2, 3, 1, 0).reshape(9, 64, 128).transpose(1, 0, 2)
    )  # [64, 9, 128]

    w1r = np.ascontiguousarray(f(inputs["w_fc1"]).reshape(128, 25, 512))
    wf2 = np.ascontiguousarray(f(inputs["w_fc2"]).reshape(4, 128, 512).transpose(1, 0, 2))
    wcls = np.ascontiguousarray(f(inputs["w_cls"]).reshape(4, 128, 16).transpose(1, 0, 2))

    col = lambda a: np.ascontiguousarray(np.asarray(a, np.float32).reshape(-1, 1))
    row = lambda a: np.ascontiguousarray(np.asarray(a, np.float32).reshape(1, -1))
    w = {
        "wqkv": np.ascontiguousarray(wqkv_ext),
        "bqkv_row": row(bqkv_ext),
        "WA": np.ascontiguousarray(WA),
        "b1": col(np.concatenate([b1p, b1p])),
        "w2t": w2t,
        "b2": col(inputs["b_ffn2"]),
        "wc1": wc1,
        "bc1": col(inputs["b_c1"]),
        "wc2": wc2,
        "bc2": col(inputs["b_c2"]),
        "w1r": w1r,
        "b1row": row(inputs["b_fc1"]),
        "wf2": wf2,
        "b2row": row(inputs["b_fc2"]),
        "wcls": wcls,
        "bcrow": row(inputs["b_cls"]),
        "eye": np.eye(128, dtype=np.float32),
        "eyebf": np.eye(128, dtype=np.float32),
        "ones1": np.ones((1, S), dtype=np.float32),
    }
    flags = {
        "qkv_bias": bool(np.any(bqkv_ext)),
        "fc1_bias": bool(np.any(w["b1row"])),
        "fc2_bias": bool(np.any(w["b2row"])),
        "cls_bias": bool(np.any(w["bcrow"])),
    }
    return w, flags


class _W:
    pass


_F32_WEIGHTS = {"b1", "b2", "bc1", "bc2"}            # activation-bias operands
_BF16_WEIGHTS = {"wqkv", "WA", "w2t", "wc1", "wc2", "eyebf"}  # bf16 matmul path


def _load_weights(nc, pool, wvals):
    """Declare dram params + DMA every weight into resident SBUF tiles.
    bf16 weights are cast during a gpsimd DMA (only engine that casts)."""
    W = _W()
    for name, arr in wvals.items():
        if name in _F32_WEIGHTS:
            dt = F32
        elif name in _BF16_WEIGHTS:
            dt = BF16
        else:
            dt = F32R
        dram = nc.declare_dram_parameter(
            name, list(arr.shape), F32 if dt == BF16 else dt, isOutput=False
        )
        t = pool.tile(list(arr.shape), dt, name=f"sb_{name}")
        if dt == BF16:
            nc.gpsimd.dma_start(out=t, in_=dram[:])
        else:
            nc.sync.dma_start(out=t, in_=dram[:])
        setattr(W, name, t)
    return W


def _win(ap, offset, dims):
    """Manual sub-AP of a tile: dims = [[stride, count], ...] free dims."""
    return bass.AP(
        tensor=ap.tensor, offset=ap.offset + offset,
        ap=[list(ap.ap[0])] + [list(d) for d in dims],
    )


def _bcast(ap, nt, width):
    """[128, NT] stat tile -> broadcast AP [128, nt, width] (0-stride inner)."""
    return bass.AP(
        tensor=ap.tensor, offset=ap.offset,
        ap=[list(ap.ap[0]), [1, nt], [0, width]],
    )


def _mm(nc, out, lhsT, rhs, start=True, stop=True):
    nc.tensor.matmul(out, lhsT, rhs, start=start, stop=stop)


def _tp(nc, out, in_, eye):
    nc.tensor.matmul(out, in_, eye, is_transpose=True)


def _s0_start(nc, pools, x_dram, s):
    """Allocate the sample's tiles + launch its x DMA."""
    acts = pools["acts"]
    st = {"s": s}
    x_s = acts.tile([C, 768], BF16, name="x_s", bufs=5)
    nc.sync.dma_start(out=x_s[:, 0:NTOK], in_=x_dram[s])
    st["x_s"] = x_s
    st["q2k2"] = acts.tile([128, NT, 256], BF16, name="q2k2", bufs=3)
    st["kvsrc"] = acts.tile([128, NT, 192], BF16, name="kvsrc", bufs=4)
    return st


def _s0_qkv(nc, pools, W, flags, st):
    """qkv in two psum halves; extracts per half (ACT square + DVE vW)."""
    psum = pools["psum"]
    x_s, q2k2, kvsrc = st["x_s"], st["q2k2"], st["kvsrc"]
    for half in range(2):
        pq = psum.tile([128, 3, 512], F32, name="pq", tag="pqkv", bufs=1)
        for i in range(3):
            t = half * 3 + i
            nt = TOK_SIZES[t]
            _mm(nc, pq[0:nt, i, 0:QW], x_s[:, 128 * t : 128 * t + nt],
                W.wqkv, start=True, stop=not flags["qkv_bias"])
            if flags["qkv_bias"]:
                _mm(nc, pq[0:nt, i, 0:QW], W.ones1[0:1, 0:nt], W.bqkv_row,
                    start=False, stop=True)
        h3 = slice(3 * half, 3 * half + 3)
        nc.scalar.activation(q2k2[:, h3, :], pq[:, :, 0:256], AF.Square)
        nc.vector.tensor_copy(kvsrc[:, h3, 0:64], pq[:, :, 256:320])


def _s0_stats(nc, pools, W, st):
    """Token stats + q2n / k2a scales (needs both qkv halves)."""
    acts, stats = pools["acts"], pools["stats"]
    q2k2, kvsrc = st["q2k2"], st["kvsrc"]
    q2 = q2k2.rearrange("p t (h c) -> p t h c", h=2)[:, :, 0, :]   # [128,NT,128]
    k2 = q2k2.rearrange("p t (h c) -> p t h c", h=2)[:, :, 1, :]

    # DVE k-chain first so k2a (pool's first op) unblocks quickly; q4 runs
    # on DVE afterwards, in parallel with the pool's k2a.
    sk2 = stats.tile([128, NT], F32, name="sk2", bufs=3)
    nc.vector.reduce_sum(sk2, k2, axis=AX.X)
    # sk2 = sum_c k^2 ~ O(5) for randn inputs; the reference's +1e-7 is
    # numerically invisible, so skip the add
    s1kb = stats.tile([128, NT], BF16, name="s1kb", bufs=3)
    with nc.allow_low_precision(reason="bf16 per-token scales, ~0.4% on attn"):
        nc.vector.reciprocal(s1kb, sk2)
    nc.gpsimd.tensor_tensor(kvsrc[:, :, 64:192], k2, _bcast(s1kb, NT, 128), ALU.mult)
    q4 = acts.tile([128, NT, 128], BF16, name="q4", bufs=2)
    nc.vector.tensor_mul(q4, q2, q2)

    sq4 = stats.tile([128, NT], F32, name="sq4", bufs=3)
    nc.vector.reduce_sum(sq4, q4, axis=AX.X)
    nq = stats.tile([128, NT], F32, name="nq", bufs=3)
    nc.scalar.activation(nq, sq4, AF.Sqrt)
    cqb = stats.tile([128, NT], BF16, name="cqb", bufs=3)
    with nc.allow_low_precision(reason="bf16 per-token scales, ~0.4% on attn"):
        nc.vector.reciprocal(cqb, nq)
    q2n = acts.tile([128, NT, 128], BF16, name="q2n", bufs=4)
    nc.gpsimd.tensor_tensor(q2n, q2, _bcast(cqb, NT, 128), ALU.mult)
    st.update(q2n=q2n, kvsrc=kvsrc)
    return st


def _s1_kv_pair(nc, pools, W, sts, pkv):
    """M = k2a^T vW and the k2a gram (column norms) in one PE pass.
    Both samples share one psum bank; gram-diag/invs chain pair-fused.
    invs lands on the msb extract (per-partition there), so q2nT stays a
    plain copy off the transpose."""
    acts, stats = pools["acts"], pools["stats"]
    npair = len(sts)
    for si, st in enumerate(sts):
        kvsrc = st["kvsrc"]
        for t in range(NT):
            nt = TOK_SIZES[t]
            _mm(nc, pkv[:, si, :], kvsrc[0:nt, t, 64:192], kvsrc[0:nt, t, :],
                start=(t == 0), stop=(t == NT - 1))
    tmpd = acts.tile([128, 2, 128], F32, name="tmpd", bufs=2)
    s2 = stats.tile([128, 2], F32, name="s2", bufs=3)
    eye_b = bass.AP(
        tensor=W.eye.tensor, offset=W.eye.offset,
        ap=[list(W.eye.ap[0]), [0, npair], [1, 128]],
    )
    nc.vector.tensor_mul(tmpd[:, 0:npair], pkv[:, 0:npair, 64:192], eye_b)
    nc.vector.reduce_sum(s2[:, 0:npair], tmpd[:, 0:npair], axis=AX.X)
    invs = stats.tile([128, 2], F32, name="invs", bufs=3)
    nc.scalar.activation(invs[:, 0:npair], s2[:, 0:npair], AF.Sqrt,
                         scale=float(NTOK))  # 27*sqrt(s2)
    nc.vector.reciprocal(invs[:, 0:npair], invs[:, 0:npair])
    for si, st in enumerate(sts):
        msb = acts.tile([128, 64], BF16, name="msb", bufs=3)
        nc.scalar.mul(msb, pkv[:, si, 0:64], invs[:, si : si + 1])
        st["msb"] = msb


def _s1_tp(nc, pools, W, st):
    """Transpose q2n to channel-major."""
    acts, psum = pools["acts"], pools["psum"]
    q2n = st["q2n"]
    pqt = psum.tile([128, 768], BF16, name="pqt", tag="ps2", bufs=2)
    for t in range(NT):
        _tp(nc, pqt[:, 128 * t : 128 * (t + 1)], q2n[:, t, :], W.eyebf)
    q2nT = acts.tile([128, 768], BF16, name="q2nT", bufs=2)
    nc.scalar.copy(q2nT, pqt)
    st["q2nT"] = q2nT


def _s2_ffn1_x(nc, pools, W, st, ph, si):
    """ffn1 x-part; the pair shares one psum tile, sample si on partition
    half si (PE tile_position col = 64*si)."""
    x_s = st["x_s"]
    lo = 64 * si
    _mm(nc, ph[lo : lo + 64, 0:512], W.WA, x_s[:, 0:512], start=True, stop=False)
    _mm(nc, ph[lo : lo + 64, 512:736], W.WA, x_s[:, 512:736], start=True, stop=False)


def _s2_ffn1_m(nc, pools, W, st, ph, si):
    q2nT, msb = st["q2nT"], st["msb"]
    lo = 64 * si
    _mm(nc, ph[lo : lo + 64, 0:512], msb, q2nT[:, 0:512], start=False, stop=True)
    _mm(nc, ph[lo : lo + 64, 512:736], msb, q2nT[:, 512:736], start=False, stop=True)


def _s2_ffn1_act(nc, pools, W, sts, ph):
    """One pair-wide prelu extract off the stacked psum tile."""
    acts = pools["acts"]
    np_ = 64 * len(sts)
    h = acts.tile([128, 736], BF16, name="h", bufs=2)
    nc.scalar.activation(h[0:np_, 0:732], ph[0:np_, 0:732], AF.Prelu,
                         bias=W.b1[0:np_], alpha=0.01)
    for si, st in enumerate(sts):
        st["h"] = h
        st["hsi"] = si


def _s2_ffn2(nc, pools, W, st):
    acts, psum = pools["acts"], pools["psum"]
    x_s, h, si = st["x_s"], st["h"], st["hsi"]
    lo = 64 * si
    pxen = psum.tile([128, 768], F32, name="pxen", tag="ps2", bufs=2)
    _mm(nc, pxen[:, 0:512], W.w2t[lo : lo + 64, :], h[lo : lo + 64, 0:512],
        start=True, stop=False)
    _mm(nc, pxen[:, 512:732], W.w2t[lo : lo + 64, :], h[lo : lo + 64, 512:732],
        start=True, stop=False)
    # residual via identity matmul: keeps the extract a plain ACT copy, off
    # the congested DVE queue (conv1's ps2 slot waits on this extract)
    _mm(nc, pxen[:, 0:512], W.eyebf, x_s[:, 0:512], start=False, stop=True)
    _mm(nc, pxen[:, 512:732], W.eyebf, x_s[:, 512:732], start=False, stop=True)
    xen = acts.tile([C, 768], BF16, name="xen", bufs=3)
    nc.scalar.activation(xen[:, 0:NTOK], pxen[:, 0:NTOK], AF.Identity, bias=W.b2)
    st["xen"] = xen


def _s2_c1(nc, pools, W, st, grp):
    """conv1 (24x24 garbage-free windows, rows split 0-11 / 12-23) + pool."""
    acts, psum = pools["acts"], pools["psum"]
    xen, s = st["xen"], st["s"]
    pc1a = psum.tile([64, 12, 24], F32, name="pc1a", tag="ps2", bufs=2)
    pc1b = psum.tile([64, 12, 24], F32, name="pc1b", tag="ps2", bufs=2)
    for ky in range(3):
        for kx in range(3):
            tap = ky * 3 + kx
            _mm(nc, pc1a, W.wc1[:, tap, :],
                _win(xen, ky * 27 + kx, [[27, 12], [1, 24]]),
                start=(tap == 0), stop=(tap == 8))
            _mm(nc, pc1b, W.wc1[:, tap, :],
                _win(xen, (ky + 12) * 27 + kx, [[27, 12], [1, 24]]),
                start=(tap == 0), stop=(tap == 8))
    o1r = acts.tile([64, 576], BF16, name="o1r", bufs=2)
    o1rv = o1r.rearrange("p (h w) -> p h w", h=24)
    nc.scalar.activation(o1rv[:, 0:12, :], pc1a, AF.Relu, bias=W.bc1)
    nc.scalar.activation(o1rv[:, 12:24, :], pc1b, AF.Relu, bias=W.bc1)
    # pool: max over w-pairs via reduce (innermost), then h-pairs via TT max
    r1 = acts.tile([64, 24, 12], BF16, name="r1", bufs=2)
    o1rw = o1r.rearrange("p (h w2 wp) -> p h w2 wp", h=24, wp=2)
    nc.vector.tensor_reduce(r1, o1rw, axis=AX.X, op=ALU.max)
    g = s % CGRP
    o1pv = grp["o1p"][:, g, 0:144].rearrange("p (a b) -> p a b", a=12)
    nc.vector.tensor_max(o1pv, r1[:, 0:24:2, :], r1[:, 1:24:2, :])


def _emit_conv2_group(nc, pools, W, O2buf, grp, g0, gn):
    """conv2+pool for a group of gn samples (moving dim = gn*120)."""
    acts, psum = pools["acts"], pools["psum"]
    pc2 = psum.tile([128, CGRP, 10, 10], F32, name="pc2", tag="ps1", bufs=1)
    for ky in range(3):
        for kx in range(3):
            tap = ky * 3 + kx
            _mm(nc, pc2[:, 0:gn], W.wc2[:, tap, :],
                _win(grp["o1p"], ky * 12 + kx, [[148, gn], [12, 10], [1, 10]]),
                start=(tap == 0), stop=(tap == 8))
    o2r = acts.tile([128, CGRP, 100], BF16, name="o2r", bufs=2)
    o2rv = o2r.rearrange("p g (h w) -> p g h w", h=10)
    nc.scalar.activation(o2rv[:, 0:gn], pc2[:, 0:gn], AF.Relu, bias=W.bc2)
    n1 = acts.tile([128, CGRP, 25], F32, name="n1", bufs=2)
    n1v = n1.rearrange("p g (a b) -> p g a b", a=5)
    n2 = acts.tile([128, CGRP, 25], F32, name="n2", bufs=2)
    n2v = n2.rearrange("p g (a b) -> p g a b", a=5)
    nc.vector.tensor_max(
        n1v[:, 0:gn], o2rv[:, 0:gn, 0:10:2, 0:10:2], o2rv[:, 0:gn, 0:10:2, 1:10:2]
    )
    nc.vector.tensor_max(
        n2v[:, 0:gn], o2rv[:, 0:gn, 1:10:2, 0:10:2], o2rv[:, 0:gn, 1:10:2, 1:10:2]
    )
    outv = (
        O2buf[:, :, g0 : g0 + gn]
        .rearrange("p a g -> p g a")
        .rearrange("p g (a b) -> p g a b", a=5)
    )
    nc.vector.tensor_max(outv, n1v[:, 0:gn], n2v[:, 0:gn])


def _emit_fc(nc, pools, W, flags, out_dram, O2buf, ns):
    psum, fc = pools["psum"], pools["fc"]
    ones = W.ones1[0:1, 0:ns]

    po3 = psum.tile([ns, 512], F32, name="po3", tag="ps1", bufs=1)
    for p in range(25):
        _mm(nc, po3, O2buf[:, p, :], W.w1r[:, p, :],
            start=(p == 0), stop=(p == 24 and not flags["fc1_bias"]))
    if flags["fc1_bias"]:
        _mm(nc, po3, ones, W.b1row, start=False, stop=True)
    o3r = fc.tile([ns, 512], F32R, name="o3r")
    nc.scalar.activation(o3r, po3, AF.Relu)

    po3t = psum.tile([128, 4, ns], F32, name="po3t", tag="ps1", bufs=1)
    for j in range(4):
        nc.tensor.matmul(
            po3t[:, j, :].bitcast(F32R), o3r[:, 128 * j : 128 * (j + 1)],
            W.eye[0:ns, 0:ns].bitcast(F32R), is_transpose=True,
        )
    o3T = fc.tile([128, 4, ns], F32R, name="o3T")
    nc.vector.tensor_copy(o3T, po3t)

    po4 = psum.tile([ns, 512], F32, name="po4", tag="ps1", bufs=1)
    for j in range(4):
        _mm(nc, po4, o3T[:, j, :], W.wf2[:, j, :],
            start=(j == 0), stop=(j == 3 and not flags["fc2_bias"]))
    if flags["fc2_bias"]:
        _mm(nc, po4, ones, W.b2row, start=False, stop=True)
    o4r = fc.tile([ns, 512], F32R, name="o4r")
    nc.scalar.activation(o4r, po4, AF.Relu)

    po4t = psum.tile([128, 4, ns], F32, name="po4t", tag="ps1", bufs=1)
    for j in range(4):
        nc.tensor.matmul(
            po4t[:, j, :].bitcast(F32R), o4r[:, 128 * j : 128 * (j + 1)],
            W.eye[0:ns, 0:ns].bitcast(F32R), is_transpose=True,
        )
    o4T = fc.tile([128, 4, ns], F32R, name="o4T")
    nc.vector.tensor_copy(o4T, po4t)

    pcls = psum.tile([ns, 512], F32, name="pcls", tag="ps1", bufs=1)
    for j in range(4):
        _mm(nc, pcls[:, 0:16], o4T[:, j, :], W.wcls[:, j, :],
            start=(j == 0), stop=(j == 3 and not flags["cls_bias"]))
    if flags["cls_bias"]:
        _mm(nc, pcls[:, 0:16], ones, W.bcrow, start=False, stop=True)
    outsb = fc.tile([ns, 16], F32, name="outsb")
    nc.vector.tensor_copy(outsb, pcls[:, 0:16])
    nc.sync.dma_start(out=out_dram[:], in_=outsb)


def build_nc(wvals, flags, n_samples=S):
    nc = bass.Bass()
    x_dram = nc.declare_dram_parameter("x", [n_samples, C, NTOK], BF16, isOutput=False)
    out_dram = nc.declare_dram_parameter("out", [n_samples, 16], F32, isOutput=True)

    with tile.TileContext(nc) as tc:
        with (
            tc.tile_pool(name="wts", bufs=1) as wts,
            tc.tile_pool(name="acts", bufs=2) as acts,
            tc.tile_pool(name="stats", bufs=3) as stats,
            tc.tile_pool(name="fc", bufs=1) as fc,
            tc.tile_pool(name="psum", bufs=1, space="PSUM") as psum,
        ):
            pools = {"acts": acts, "stats": stats, "psum": psum, "fc": fc}
            W = _load_weights(nc, wts, wvals)
            O2buf = fc.tile([128, 25, n_samples], F32R, name="O2buf")
            grp = {}  # group-index -> {"o1p": tile}

            def tail_stages(sts):
                """Pair-i tail as 4 closures; the next pair's qkv halves are
                woven between them so psum-extract waits hide behind other
                matmul bursts (and vice versa)."""
                def t0():
                    pkv = psum.tile([128, 2, 192], F32, name="pkv", tag="ps1", bufs=1)
                    _s1_kv_pair(nc, pools, W, sts, pkv)
                    for st in sts:
                        _s1_tp(nc, pools, W, st)

                def t1():
                    ph = psum.tile([128, 768], F32, name="ph", tag="ps2", bufs=2)
                    for si, st in enumerate(sts):
                        _s2_ffn1_x(nc, pools, W, st, ph, si)
                    for si, st in enumerate(sts):
                        _s2_ffn1_m(nc, pools, W, st, ph, si)
                    _s2_ffn1_act(nc, pools, W, sts, ph)

                def t2():
                    for st in sts:
                        _s2_ffn2(nc, pools, W, st)

                def t3():
                    for st in sts:
                        _s2_c1(nc, pools, W, st, grp[st["s"] // CGRP])
                    s_last = sts[-1]["s"]
                    if s_last % CGRP == CGRP - 1 or s_last == n_samples - 1:
                        g0 = (s_last // CGRP) * CGRP
                        _emit_conv2_group(
                            nc, pools, W, O2buf, grp[g0 // CGRP], g0, s_last - g0 + 1
                        )

                return [t0, t1, t2, t3]

            prev = None
            for p0 in range(0, n_samples, 2):
                pair = [p0] + ([p0 + 1] if p0 + 1 < n_samples else [])
                sts = []
                for s in pair:
                    if s % CGRP == 0:
                        grp[s // CGRP] = {
                            "o1p": acts.tile([64, CGRP, 148], BF16, name="o1p_grp", bufs=3)
                        }
                    st = _s0_start(nc, pools, x_dram, s)
                    _s0_qkv(nc, pools, W, flags, st)
                    _s0_stats(nc, pools, W, st)
                    sts.append(st)
                if prev is not None:
                    for t in tail_stages(prev):
                        t()
                prev = sts
            for t in tail_stages(prev):
                t()
            _emit_fc(nc, pools, W, flags, out_dram, O2buf, n_samples)

    _split_waits(nc)
    return nc


_BUILD_CACHE = {}


def _make_in_maps(inputs, wvals):
    bf = mybir.dt.np(BF16)
    x = np.ascontiguousarray(np.asarray(inputs["x"], np.float32)).reshape(
        N_CORES, S, C, NTOK
    ).astype(bf)
    in_maps = []
    for c in range(N_CORES):
        m = {"x": np.ascontiguousarray(x[c])}
        m.update(wvals)
        in_maps.append(m)
    return in_maps


def kernel(**inputs):
    wvals, flags = _prep_weights(inputs)
    key = tuple(sorted(flags.items()))
    if key not in _BUILD_CACHE:
        _BUILD_CACHE[key] = build_nc(wvals, flags)
    nc = _BUILD_CACHE[key]

    in_maps = _make_in_maps(inputs, wvals)
    last_err = None
    for _attempt in range(3):
        try:
            res = run_bass_kernel_spmd(nc, in_maps, core_ids=list(range(N_CORES)))
            break
        except Exception as e:  # transient device faults: retry
            last_err = e
    else:
        raise last_err
    out = np.concatenate([res.results[c]["out"] for c in range(N_CORES)], axis=0)
    return out.astype(np.float32)
